# revision 59
# baseline (speedup 1.0000x reference)
"""ASTGCN block Trainium2 kernel (v2).

Strategy: 8 cores; core c handles batch b = c//2, time-half h = c%2 (8 output
timesteps each, data-parallel over B and T). Attention (temporal Et, spatial
S) is per-b and replicated on the 2 cores sharing a b. The sparse graph
propagation is reformulated as dense (N,N) matmuls: the edge-scatter of the
symmetric norm is accumulated host-side into a dense W (the +I/-I self-loop
terms cancel), so  prop1(h) = (W*S) @ h  and  prop2(h) = W @ h.

v2 changes vs baseline:
- Input DMAs ordered by first use (Pb/Pf/Xn first) and X tensors split in
  halves so attention matmuls start ~5us in instead of after all loads.
- Single activation-table regime: sigmoid via tanh (0.5*tanh(x/2)+0.5, in
  the exp table) and LN rstd via exp(-0.5*ln(var+eps)); only one table
  switch in the whole program (exp_and_others -> natural_log_exp...).
- LayerNorm runs in pair layout: per-pair stats via ones-block matmuls on
  PE (reduce over the f partition rows), rstd/-mu*rstd broadcast back with
  block matmuls; no transposes of the conv output at all.
- Output stored in pair layout as bf16; host does the final (f,n) -> (n,f)
  transpose and fp32 upcast.
- cheb -> conv -> LN -> store software-pipelined across the 5 timestep
  pairs to keep PE dense (p-state) and overlap store DMAs with compute.

Per-core time axis is PERMUTED so the program is identical SPMD: slot t' maps
to global t via tmap (identity for h=0, rotated by 6 for h=1); all
t-dependent weights (be, Ve, Ws1, UW) are permuted host-side to match.
"""

import numpy as np

B, N, F, T = 4, 512, 64, 16
P = 128
CH = N // P            # 4 n-chunks
NSLOT = 10             # cheb window timesteps per core (5 pairs)
NP = NSLOT // 2        # 5 pairs
LN_EPS = 1e-5

PBW = 1735             # packed bf16 constant width
PFW = 148              # packed f32 constant width

_CACHE = {}


def _build_program():
    import sys
    if '/opt/trn_rl_repo' not in sys.path:
        sys.path.insert(0, '/opt/trn_rl_repo')
    from contextlib import ExitStack
    import concourse.bass as bass
    import concourse.tile as tile
    from concourse import bacc, mybir

    dt = mybir.dt
    AL = mybir.AluOpType
    AF = mybir.ActivationFunctionType
    AX = mybir.AxisListType
    f32 = dt.float32
    bf16 = dt.bfloat16

    nc = bacc.Bacc("TRN2", target_bir_lowering=False, debug=False, num_devices=1)

    def din(name, shape, d=bf16):
        return nc.dram_tensor(name, list(shape), d, kind="ExternalInput").ap()

    XnD   = din("Xn", (N, T * F))
    XwD   = din("Xw", (8, P, N))
    UWD   = din("UW", (8, P, 48))
    bshD  = din("bsh", (N, N))          # 0.5 * bs
    VsTD  = din("VsT", (N, N))
    WTD   = din("WT", (N, N))
    WpkD  = din("Wpk", (7, P, P))
    PbD   = din("Pb", (P, PBW))
    PfD   = din("Pf", (P, PFW), f32)
    ZoutD = nc.dram_tensor("Zout", [NP * P, N], bf16, kind="ExternalOutput").ap()

    with tile.TileContext(nc) as tc, ExitStack() as ctx:
        sg = ctx.enter_context(tc.tile_pool(name="sg", bufs=1))
        big = ctx.enter_context(tc.tile_pool(name="big", bufs=5, space="PSUM"))
        sml = ctx.enter_context(tc.tile_pool(name="sml", bufs=2, space="PSUM"))
        hlf = ctx.enter_context(tc.tile_pool(name="hlf", bufs=1, space="PSUM"))
        xhp = ctx.enter_context(tc.tile_pool(name="xhp", bufs=7))
        txp = ctx.enter_context(tc.tile_pool(name="txp", bufs=5))
        lnp = ctx.enter_context(tc.tile_pool(name="lnp", bufs=5))

        # ------------- input DMAs, ordered by first use -------------
        Pb = sg.tile([P, PBW], bf16, tag="pb")
        nc.sync.dma_start(out=Pb[:], in_=PbD)
        Pf = sg.tile([P, PFW], f32, tag="pf")
        nc.sync.dma_start(out=Pf[:], in_=PfD)
        XnA = sg.tile([P, 2, T * F], bf16, tag="xna")
        XnB = sg.tile([P, 2, T * F], bf16, tag="xnb")
        XnDr = XnD.rearrange("(k p) t -> p k t", k=CH)
        UWAll = sg.tile([P, 8, 48], bf16, tag="uwall")
        XwA = sg.tile([P, 4, N], bf16, tag="xwa")
        XwB = sg.tile([P, 4, N], bf16, tag="xwb")
        XwDr = XwD.rearrange("s p n -> p s n")
        # interleave the two X layouts so both attention input paths
        # (lhs0 over Xn, R48 over Xw) can start on half the data
        nc.sync.dma_start(out=XnA[:], in_=XnDr[:, 0:2, :])
        nc.sync.dma_start(out=UWAll[:], in_=UWD.rearrange("s p n -> p s n"))
        nc.sync.dma_start(out=XwA[:], in_=XwDr[:, 0:4, :])
        nc.sync.dma_start(out=XnB[:], in_=XnDr[:, 2:4, :])
        nc.sync.dma_start(out=XwB[:], in_=XwDr[:, 4:8, :])
        bsAll = sg.tile([P, CH, N], bf16, tag="bsall")
        nc.sync.dma_start(out=bsAll[:], in_=bshD.rearrange("(k p) n -> p k n", k=CH))
        VsTAll = sg.tile([P, CH, N], bf16, tag="vstall")
        nc.sync.dma_start(out=VsTAll[:], in_=VsTD.rearrange("(k p) n -> p k n", k=CH))
        WTAll = sg.tile([P, CH, N], bf16, tag="wtall")
        nc.sync.dma_start(out=WTAll[:], in_=WTD.rearrange("(k p) n -> p k n", k=CH))
        Wpk = sg.tile([P, 7, P], bf16, tag="wpk")
        nc.sync.dma_start(out=Wpk[:], in_=WpkD.rearrange("w p c -> p w c"))

        Xn = [XnA[:, 0, :], XnA[:, 1, :], XnB[:, 0, :], XnB[:, 1, :]]
        Xw = [XwA[:, s, :] for s in range(4)] + [XwB[:, s, :] for s in range(4)]
        UW = [UWAll[:, s, :] for s in range(8)]
        bsh = [bsAll[:, k, :] for k in range(CH)]
        VsT = [VsTAll[:, k, :] for k in range(CH)]
        WT = [WTAll[:, k, :] for k in range(CH)]
        WcP = [Wpk[:, k, :] for k in range(3)]
        Lprev, Lmid, Lnext, WrP = (Wpk[:, 3, :], Wpk[:, 4, :], Wpk[:, 5, :],
                                   Wpk[:, 6, :])
        # packed bf16 layout
        U1r = Pb[:, 0:4]
        Ws2d = Pb[:, 4:20]
        VeT = Pb[0:16, 20:36]
        Ws1 = Pb[0:16, 36:37]
        ones1 = Pb[0:1, 37:165]
        I128b = Pb[:, 165:293]
        U2 = Pb[0:64, 293:805]
        I16r = Pb[0:1, 805:1061]     # I16 rows flattened: e_t = [0:1, 16t:16t+16]
        B2 = Pb[:, 1061:1063]        # (128,2) block col-indicator * 1/64
        B2T = Pb[0:2, 1063:1191]     # (2,128) block row-indicator * gamma[f]
        B2T32 = Pb[32:34, 1063:1191]  # same rows replicated at partition 32
        hcVe = Pb[0:1, 1191:1207]    # 0.5*colsum(VeT')  [sigmoid-fold row]
        vch = Pb[0:1, 1207:1719]     # 0.5*colsum(VsT')  [sigmoid-fold row]
        bePb = Pb[0:16, 1719:1735]   # 0.5*be (permuted), bf16
        # packed f32 layout
        gamP = Pf[:, 0:1]
        betP = Pf[:, 1:2]
        bch = Pf[:, 2:3]
        btr = Pf[:, 3:4]
        I128f = Pf[:, 4:132]
        bePh = Pf[0:16, 132:148]     # 0.5 * be (permuted)

        zerot = sg.tile([P, N], bf16, tag="zerot")
        nc.vector.memset(zerot[:], 0.0)
        epsP = sg.tile([P, 1], f32, tag="epsP")
        nc.vector.memset(epsP[:], LN_EPS)

        # persistent sbuf intermediates
        G = [sg.tile([P, N], bf16, tag=f"g{k}", name=f"g{k}") for k in range(CH)]
        Ex = [sg.tile([P, N], bf16, tag=f"ex{k}", name=f"ex{k}") for k in range(CH)]
        A1T = [sg.tile([P, N], bf16, tag=f"a1t{k}", name=f"a1t{k}") for k in range(CH)]
        dSv = [sg.tile([P, 1], f32, tag=f"dsv{k}", name=f"dsv{k}") for k in range(CH)]
        Tx0n = [sg.tile([P, T * F], bf16, tag=f"tx0n{k}", name=f"tx0n{k}")
                for k in range(CH)]
        dSB = sg.tile([P, N], bf16, tag="dsb")

        # =====================================================
        # Attention phase
        # =====================================================
        # ---- lhs0[(t,f)] = sum_n U1[n] X[n,(t,f)]  -> (1,1024)
        # accumulation interleaved with the R48 first half so PE follows the
        # XnA / XwA / XnB / XwB DMA arrival order
        L0a = sml.tile([1, 512], f32, tag="sml", name="l0a")
        L0b = sml.tile([1, 512], f32, tag="sml", name="l0b")
        R48p = hlf.tile([48, N], f32, tag="hlf", name="r48")
        for k in range(2):
            nc.tensor.matmul(L0a[:], U1r[:, k:k + 1], Xn[k][:, 0:512],
                             start=(k == 0), stop=False)
        for k in range(2):
            nc.tensor.matmul(L0b[:], U1r[:, k:k + 1], Xn[k][:, 512:1024],
                             start=(k == 0), stop=False)
        for s in range(4):
            nc.tensor.matmul(R48p[:], UW[s][:, :], Xw[s][:, :],
                             start=(s == 0), stop=False)
        for k in range(2, CH):
            nc.tensor.matmul(L0a[:], U1r[:, k:k + 1], Xn[k][:, 0:512],
                             start=False, stop=(k == CH - 1))
        for k in range(2, CH):
            nc.tensor.matmul(L0b[:], U1r[:, k:k + 1], Xn[k][:, 512:1024],
                             start=False, stop=(k == CH - 1))
        lhs0row = sg.tile([1, T * F], bf16, tag="lhs0row")
        nc.vector.tensor_copy(lhs0row[:, 0:512], L0a[:])
        nc.vector.tensor_copy(lhs0row[:, 512:1024], L0b[:])
        # reshape to (64,16) via 16 rank-1 matmuls against identity rows
        l0Fp = sml.tile([F, T], f32, tag="sml", name="l0fp")
        for t in range(T):
            nc.tensor.matmul(l0Fp[:], lhs0row[0:1, 64 * t:64 * t + 64],
                             I16r[0:1, 16 * t:16 * t + 16],
                             start=(t == 0), stop=(t == T - 1))
        # 0.5 sigmoid-prefactor folded here: scales lhs2T and hence P0
        lhs0F = sg.tile([F, T], bf16, tag="lhs0f")
        nc.vector.tensor_scalar(lhs0F[:], l0Fp[:], 0.5, None, op0=AL.mult)

        # ---- lhs2T chunks (n,16) = U2[:,chunk].T @ lhs0F, packed in one psum
        l2p = sml.tile([P, CH * T], f32, tag="sml", name="l2t")
        for k in range(CH):
            nc.tensor.matmul(l2p[:, k * T:(k + 1) * T],
                             U2[:, k * P:(k + 1) * P], lhs0F[:],
                             start=True, stop=True)
        l2s = sg.tile([P, CH * T], bf16, tag="l2ts")
        nc.vector.tensor_copy(l2s[:], l2p[:])
        lhs2T = [l2s[:, k * T:(k + 1) * T] for k in range(CH)]

        # ---- R48 second half: rows 0:16 rhs3T (Ws3), rows 32:48 rhs_tT (U3)
        for s in range(4, 8):
            nc.tensor.matmul(R48p[:], UW[s][:, :], Xw[s][:, :],
                             start=False, stop=(s == 7))
        R48 = sg.tile([48, N], bf16, tag="r48s")
        nc.scalar.copy(R48[:], R48p[:])

        # ---- rhs_tn chunks: transpose R48[32:48], packed in one psum
        rtp = hlf.tile([P, CH * T], bf16, tag="hlf", name="rtn")
        for k in range(CH):
            nc.tensor.transpose(rtp[:, k * T:(k + 1) * T],
                                R48[32:48, k * P:(k + 1) * P],
                                I128b[32:48, 32:48])
        rts = sg.tile([P, CH * T], bf16, tag="rtns")
        nc.vector.tensor_copy(rts[:], rtp[:])
        rhs_tn = [rts[:, k * T:(k + 1) * T] for k in range(CH)]

        # ---- P0 (16,16) = 0.5*(lhs_t @ rhs_t) + 0.5*be (bias via I16 matmul)
        P0p = sml.tile([T, T], f32, tag="sml", name="p0")
        for k in range(CH):
            nc.tensor.matmul(P0p[:], lhs2T[k][:], rhs_tn[k][:],
                             start=(k == 0), stop=False)
        nc.tensor.matmul(P0p[:], I128b[0:16, 0:16], bePb[:],
                         start=False, stop=True)
        # sig holds tanh(0.5*(P0+be)); sigmoid affine folded into E1T matmul
        # (VeT is 0.5-scaled host-side, hcVe rank-1 term)
        sig = sg.tile([T, T], bf16, tag="sig")
        nc.scalar.activation(sig[:], P0p[:], AF.Tanh)

        # ---- E1^T = sigmoid^T @ Ve^T ; softmax over free dim
        E1Tp = sml.tile([T, T], f32, tag="sml", name="e1t")
        nc.tensor.matmul(E1Tp[:], sig[:], VeT[:], start=True, stop=False)
        nc.tensor.matmul(E1Tp[:], ones1[0:1, 0:16], hcVe[:],
                         start=False, stop=True)
        # values are O(1e-1): skip the max-subtraction for softmax
        sume = sg.tile([T, 1], f32, tag="sume")
        EtT = sg.tile([T, T], bf16, tag="ett")
        nc.scalar.activation(EtT[:], E1Tp[:], AF.Exp,
                             scale=1.0, accum_out=sume[:, 0:1])
        rse = sg.tile([T, 1], f32, tag="rse")
        nc.vector.reciprocal(rse[:], sume[:])
        nc.vector.tensor_scalar(EtT[:], EtT[:], rse[:, 0:1], None, op0=AL.mult)
        Etp = hlf.tile([T, T], bf16, tag="hlf", name="etp")
        nc.tensor.transpose(Etp[:], EtT[:], I128b[0:16, 0:16])
        Et = sg.tile([T, T], bf16, tag="et")
        nc.vector.tensor_copy(Et[:], Etp[:])

        # ---- w1e row (1,16) = Ws1.T @ EtT ; broadcast straight into the
        # pair layout w1Bpair[p=(v,f), s] = w1e[2s+v] with two base-64 matmuls
        w1p = sml.tile([1, T], f32, tag="sml", name="w1p")
        nc.tensor.matmul(w1p[:], Ws1[:], EtT[:], start=True, stop=True)
        w1row = sg.tile([1, T], bf16, tag="w1row")
        nc.scalar.copy(w1row[:], w1p[:])
        w1B2p = sml.tile([P, 8], f32, tag="sml", name="w1b2p")
        nc.tensor.matmul(w1B2p[0:64, :], ones1[0:1, 0:64],
                         w1row[:, 0:T:2], start=True, stop=True)
        nc.tensor.matmul(w1B2p[64:128, :], ones1[0:1, 0:64],
                         w1row[:, 1:T:2], start=True, stop=True)
        # ---- Ws2wP[p=(v,f), s, t] = Ws2d[p,t] * w1e[2s+v]   (128, 8, 16)
        # (reads the w1e broadcast straight from PSUM, saves a copy hop)
        Ws2w = sg.tile([P, 8, T], bf16, tag="ws2w")
        nc.vector.tensor_tensor(
            Ws2w[:],
            Ws2d[:].unsqueeze(1).broadcast_to((P, 8, T)),
            w1B2p[:].unsqueeze(2).broadcast_to((P, 8, T)),
            op=AL.mult)

        # ---- lhs_sT (16, 512) = sum_t1 (Ws2*w1e[t1]).T @ X^T[t1]
        lsTp = sml.tile([T, N], f32, tag="sml", name="lst")
        for s in range(8):
            nc.tensor.matmul(lsTp[:], Ws2w[:, s, :], Xw[s][:, :],
                             start=(s == 0), stop=(s == 7))
        lsT = sg.tile([T, N], bf16, tag="lsts")
        nc.scalar.copy(lsT[:], lsTp[:])

        # ---- rhs_s (16, 512) = Et-weighted rhs3; 0.5 sigmoid-prefactor folded
        rsp = sml.tile([T, N], f32, tag="sml", name="rsp")
        nc.tensor.matmul(rsp[:], Et[:], R48[0:16, :], start=True, stop=True)
        rss = sg.tile([T, N], bf16, tag="rss")
        nc.scalar.mul(rss[:], rsp[:], 0.5)

        # ---- P chunks; bs bias absorbed via identity matmul; G holds
        # tanh(0.5*P + bsh); the sigmoid affine is folded into M1T
        # (VsT 0.5-scaled host-side + vch rank-1 term)
        for k in range(CH):
            Pp = big.tile([P, N], f32, tag="big", name="pp")
            nc.tensor.matmul(Pp[:], lsT[:, k * P:(k + 1) * P], rss[:],
                             start=True, stop=False)
            nc.tensor.matmul(Pp[:], I128b[:], bsh[k][:],
                             start=False, stop=True)
            nc.scalar.activation(G[k][:], Pp[:], AF.Tanh)

        # ---- M1T chunks (c-part, r) + masked softmax -> A1T, dS
        for c in range(CH):
            Mp = big.tile([P, N], f32, tag="big", name="mp")
            for k in range(CH):
                nc.tensor.matmul(Mp[:], G[k][:, c * P:(c + 1) * P], VsT[k][:],
                                 start=(k == 0), stop=False)
            nc.tensor.matmul(Mp[:], ones1[0:1, 0:128], vch[:],
                             start=False, stop=True)
            sme = sg.tile([P, 1], f32, tag=f"sme{c}", name=f"sme{c}")
            nc.scalar.activation(Ex[c][:], Mp[:], AF.Exp,
                                 scale=1.0, accum_out=sme[:, 0:1])
            rcp = sg.tile([P, 1], f32, tag=f"rcp{c}", name=f"rcp{c}")
            nc.vector.reciprocal(rcp[:], sme[:])
            # A1T = (Ex * rcp) * WT   (= S^T o W^T)
            nc.vector.scalar_tensor_tensor(A1T[c][:], Ex[c][:], rcp[:, 0:1],
                                           WT[c][:], op0=AL.mult, op1=AL.mult)
            # diag: dS = sum_r (Ex*rcp)*I over the diagonal block
            dtmp = sg.tile([P, P], bf16, tag="dtmp")
            nc.vector.scalar_tensor_tensor(dtmp[:], Ex[c][:, c * P:(c + 1) * P],
                                           rcp[:, 0:1], I128b[:],
                                           op0=AL.mult, op1=AL.mult)
            nc.vector.tensor_reduce(dSv[c][:], dtmp[:], axis=AX.X, op=AL.add)

        # ---- dS row + broadcast tile (128, 512)
        dSrp = sml.tile([1, N], f32, tag="sml", name="dsrp")
        for c in range(CH):
            nc.tensor.transpose(dSrp[:, c * P:(c + 1) * P], dSv[c][:], I128f[:])
        dSrow = sg.tile([1, N], bf16, tag="dsrow")
        nc.scalar.copy(dSrow[:], dSrp[:])
        dSBp = sml.tile([P, N], f32, tag="sml", name="dsbp")
        nc.tensor.matmul(dSBp[:], ones1[:], dSrow[:], start=True, stop=True)
        nc.scalar.copy(dSB[:], dSBp[:])

        # ---- Tx0 in n-layout (all t at once)
        for k in range(CH):
            nc.vector.tensor_scalar(Tx0n[k][:], Xn[k][:], dSv[k][:, 0:1], None,
                                    op0=AL.mult)

        # =====================================================
        # Cheb + conv + LN, software-pipelined per pair
        # =====================================================
        Tx0P = {}
        TAp = {}
        Tx1T = {}
        ptA = {}
        Tx1n = {}
        TBp = {}
        Tx2T = {}
        TCp = {}
        XhP = {-1: zerot, NP: zerot}
        TDp = {}
        ZT = {}

        def e_tx0p(q):
            t = txp.tile([P, N], bf16, tag="tx0p", name=f"tx0p{q}")
            nc.gpsimd.tensor_tensor(t[:], Xw[q][:], dSB[:], op=AL.mult)
            Tx0P[q] = t

        def e_ta(q):
            p = big.tile([P, N], f32, tag="big", name=f"ta{q}")
            for k in range(CH):
                lhs = Tx0n[k][:, 2 * q * F:(2 * q + 2) * F]
                nc.tensor.matmul(p[:], lhs, A1T[k][:],
                                 start=(k == 0), stop=(k == CH - 1))
            TAp[q] = p

        def e_b(q):
            t = txp.tile([P, N], bf16, tag="tx1t", name=f"tx1t{q}")
            nc.vector.tensor_copy(t[:], TAp[q][:])
            Tx1T[q] = t

        def e_c(q):
            p = hlf.tile([P, N], bf16, tag="hlf", name=f"pta{q}")
            for k in range(CH):
                nc.tensor.transpose(p[:, k * P:(k + 1) * P],
                                    Tx1T[q][:, k * P:(k + 1) * P], I128b[:])
            ptA[q] = p

        def e_d(q):
            t = txp.tile([P, N], bf16, tag="tx1n", name=f"tx1n{q}")
            nc.scalar.copy(t[:], ptA[q][:])
            Tx1n[q] = t

        def e_e(q):
            p = big.tile([P, N], f32, tag="big", name=f"tb{q}")
            for k in range(CH):
                nc.tensor.matmul(p[:], Tx1n[q][:, k * P:(k + 1) * P], WT[k][:],
                                 start=(k == 0), stop=(k == CH - 1))
            TBp[q] = p

        def e_f(q):
            t = txp.tile([P, N], bf16, tag="tx2t", name=f"tx2t{q}")
            nc.vector.scalar_tensor_tensor(t[:], TBp[q][:], 2.0, Tx0P[q][:],
                                           op0=AL.mult, op1=AL.subtract)
            Tx2T[q] = t

        def e_g(q):
            p = big.tile([P, N], f32, tag="big", name=f"tc{q}")
            nc.tensor.matmul(p[:], WcP[0][:], Tx0P[q][:], start=True, stop=False)
            nc.tensor.matmul(p[:], WcP[1][:], Tx1T[q][:], start=False, stop=False)
            nc.tensor.matmul(p[:], WcP[2][:], Tx2T[q][:], start=False, stop=True)
            TCp[q] = p

        def e_h(q):
            t = xhp.tile([P, N], bf16, tag="xh", name=f"xh{q}")
            nc.scalar.activation(t[:], TCp[q][:], AF.Relu, bias=bch[:, 0:1],
                                 scale=1.0)
            XhP[q] = t

        def e_i(q):
            p = big.tile([P, N], f32, tag="big", name=f"td{q}")
            nc.tensor.matmul(p[:], Lprev[:], XhP[q - 1][:], start=True, stop=False)
            nc.tensor.matmul(p[:], Lmid[:], XhP[q][:], start=False, stop=False)
            nc.tensor.matmul(p[:], Lnext[:], XhP[q + 1][:], start=False, stop=False)
            nc.tensor.matmul(p[:], WrP[:], Xw[q][:], start=False, stop=True)
            TDp[q] = p

        def e_j(q):
            t = lnp.tile([P, N], bf16, tag="zt", name=f"zt{q}")
            nc.scalar.activation(t[:], TDp[q][:], AF.Relu, bias=btr[:, 0:1],
                                 scale=1.0)
            ZT[q] = t

        RSTD = {}
        NMR = {}
        SQ = {}

        def e_sq(q):
            t = lnp.tile([P, N], bf16, tag="sq", name=f"sq{q}")
            nc.vector.tensor_tensor(t[:], ZT[q][:], ZT[q][:], op=AL.mult)
            SQ[q] = t

        def e_ln1(qa, qb):
            # batched stats for a PAIR GROUP: pair qa in rows 0:2, pair qb in
            # rows 32:34 (matmul out base partition must be 0/32/64). One
            # small-op chain then serves two timestep pairs; rows 2:32 are
            # never-read garbage.
            rows = 34 if qb is not None else 2
            sA = sml.tile([34, N], f32, tag="sml", name=f"sA{qa}")
            sB = sml.tile([34, N], f32, tag="sml", name=f"sB{qa}")
            nc.tensor.matmul(sA[0:2, :], B2[:], ZT[qa][:], start=True, stop=True)
            nc.tensor.matmul(sB[0:2, :], B2[:], SQ[qa][:], start=True, stop=True)
            if qb is not None:
                nc.tensor.matmul(sA[32:34, :], B2[:], ZT[qb][:],
                                 start=True, stop=True)
                nc.tensor.matmul(sB[32:34, :], B2[:], SQ[qb][:],
                                 start=True, stop=True)
            mu = lnp.tile([34, N], f32, tag="mu", name=f"mu{qa}")
            nc.scalar.copy(mu[0:rows, :], sA[0:rows, :])
            mu2 = lnp.tile([34, N], f32, tag="mu2", name=f"mu2{qa}")
            # Pool while the pipe is full (throughput), DVE in the drain
            # (latency: Pool elementwise runs at 0.42 efficiency)
            eng = nc.gpsimd if qa == 0 else nc.vector
            eng.tensor_tensor(mu2[0:rows, :], mu[0:rows, :], mu[0:rows, :],
                              op=AL.mult)
            # var+eps = (msq + eps) - mu^2 in one stt
            var = lnp.tile([34, N], f32, tag="var", name=f"var{qa}")
            nc.vector.scalar_tensor_tensor(var[0:rows, :], sB[0:rows, :],
                                           LN_EPS, mu2[0:rows, :],
                                           op0=AL.add, op1=AL.subtract)
            # rstd = sqrt(1/(var+eps)); approx recip is ~18 bits, plenty
            rv = lnp.tile([34, N], f32, tag="rv", name=f"rv{qa}")
            nc.vector.reciprocal_approx_fast(rv[0:rows, :], var[0:rows, :])
            rstd = lnp.tile([34, N], bf16, tag="rstd", name=f"rstd{qa}")
            nc.scalar.activation(rstd[0:rows, :], rv[0:rows, :], AF.Sqrt)
            # nmr = mu * rstd
            nmr = lnp.tile([34, N], bf16, tag="nmr", name=f"nmr{qa}")
            eng.tensor_tensor(nmr[0:rows, :], mu[0:rows, :], rstd[0:rows, :],
                              op=AL.mult)
            RSTD[qa] = rstd[0:2, :]
            NMR[qa] = nmr[0:2, :]
            if qb is not None:
                RSTD[qb] = rstd[32:34, :]
                NMR[qb] = nmr[32:34, :]

        def e_ln2(q):
            z = ZT[q]
            # broadcast to 128 partitions; gamma[f] is folded into B2T
            bt = B2T if q % 2 == 0 else B2T32
            rBp = big.tile([P, N], f32, tag="big", name=f"rbp{q}")
            nc.tensor.matmul(rBp[:], bt[:], RSTD[q][:], start=True, stop=True)
            nBp = big.tile([P, N], f32, tag="big", name=f"nbp{q}")
            nc.tensor.matmul(nBp[:], bt[:], NMR[q][:], start=True, stop=True)
            # w = z*(rstd*gam)B + bet - (mu*rstd*gam)B
            u = lnp.tile([P, N], bf16, tag="u", name=f"u{q}")
            nc.vector.tensor_tensor(u[:], z[:], rBp[:], op=AL.mult)
            w = lnp.tile([P, N], bf16, tag="w", name=f"w{q}")
            nc.vector.scalar_tensor_tensor(w[:], u[:], betP[:, 0:1], nBp[:],
                                           op0=AL.add, op1=AL.subtract)
            nc.sync.dma_start(out=ZoutD[q * P:(q + 1) * P, :], in_=w[:])

        # pipeline drive, depth 7: every PE group's inputs are produced in a
        # PREVIOUS iteration, so the in-order PE queue never head-of-line
        # blocks. Stage distances: TA@0, trans@1, TB@2, TC@3, TD@4, ln1@5,
        # ln2@7. Cross-engine hops (stt, relu, copies) happen within the
        # iteration that produced their psum input.
        def live(q):
            return 0 <= q < NP

        for i in range(NP + 7):
            if live(i - 7):
                e_ln2(i - 7)
            if live(i - 1):
                e_c(i - 1)
            if live(i):
                if i == 0:
                    e_tx0p(0)
                e_ta(i)
            if live(i - 1):
                e_d(i - 1)
            if live(i - 2):
                e_e(i - 2)
                e_f(i - 2)
            if live(i - 3):
                e_g(i - 3)
                e_h(i - 3)
            if live(i):
                e_b(i)
                if live(i + 1):
                    e_tx0p(i + 1)
            if live(i - 4):
                e_i(i - 4)
                e_j(i - 4)
                e_sq(i - 4)
            # grouped stats chains: (0,1) @ i=6, (2,3) @ i=8, (4) @ i=9
            if i == 6:
                e_ln1(0, 1)
            elif i == 8:
                e_ln1(2, 3)
            elif i == 9:
                e_ln1(4, None)

    nc.compile()
    return nc


def _host_prep(inputs):
    import ml_dtypes
    bf = ml_dtypes.bfloat16

    X = np.asarray(inputs['X'], np.float32)
    edge_index = np.asarray(inputs['edge_index'])
    U1 = np.asarray(inputs['U1'], np.float32)
    U2 = np.asarray(inputs['U2'], np.float32)
    U3 = np.asarray(inputs['U3'], np.float32)
    be = np.asarray(inputs['be'], np.float32)
    Ve = np.asarray(inputs['Ve'], np.float32)
    Ws1 = np.asarray(inputs['Ws1'], np.float32)
    Ws2 = np.asarray(inputs['Ws2'], np.float32)
    Ws3 = np.asarray(inputs['Ws3'], np.float32)
    bs = np.asarray(inputs['bs'], np.float32)
    Vs = np.asarray(inputs['Vs'], np.float32)
    W_cheb = np.asarray(inputs['W_cheb'], np.float32)
    b_cheb = np.asarray(inputs['b_cheb'], np.float32)
    Wt = np.asarray(inputs['Wt'], np.float32)
    bt = np.asarray(inputs['bt'], np.float32)
    Wr = np.asarray(inputs['Wr'], np.float32)
    br = np.asarray(inputs['br'], np.float32)
    gamma = np.asarray(inputs['gamma'], np.float32)
    beta = np.asarray(inputs['beta'], np.float32)

    # dense symmetric-norm matrix (self-loop +I/-I terms cancel)
    row, col = edge_index[0].astype(np.int64), edge_index[1].astype(np.int64)
    deg = np.zeros(N, np.float32)
    np.add.at(deg, row, 1.0)
    dis = np.where(deg > 0, 1.0 / np.sqrt(np.maximum(deg, 1.0)), 0.0).astype(np.float32)
    wn = -dis[row] * dis[col]
    W = np.zeros((N, N), np.float32)
    np.add.at(W, (row, col), wn)

    # conv block matrices: L[(v,fi),(u,fo)] = Wt[fo,fi,0,dt]
    WtT = [np.ascontiguousarray(Wt[:, :, 0, d].T) for d in range(3)]  # (fi,fo)
    Z64 = np.zeros((F, F), np.float32)
    Lmid = np.block([[WtT[1], WtT[0]], [WtT[2], WtT[1]]]).astype(bf)
    Lprev = np.block([[Z64, Z64], [WtT[0], Z64]]).astype(bf)
    Lnext = np.block([[Z64, WtT[2]], [Z64, Z64]]).astype(bf)
    WrT = np.ascontiguousarray(Wr[:, :, 0, 0].T)
    WrP = np.block([[WrT, Z64], [Z64, WrT]]).astype(bf)
    WcP = np.stack([np.block([[W_cheb[k], Z64], [Z64, W_cheb[k]]]) for k in range(3)]
                   ).astype(bf)

    Wpk = np.stack([WcP[0], WcP[1], WcP[2], Lprev, Lmid, Lnext, WrP])

    Pf = np.zeros((P, PFW), np.float32)
    Pf[:, 0] = np.tile(gamma, 2)
    Pf[:, 1] = np.tile(beta, 2)
    Pf[:, 2] = np.tile(b_cheb, 2)
    Pf[:, 3] = np.tile(bt + br, 2)
    Pf[:, 4:132] = np.eye(P, dtype=np.float32)

    VsTh = 0.5 * np.ascontiguousarray(Vs.T)
    vch = VsTh.sum(axis=0)                 # 0.5*colsum(Vs^T) sigmoid-fold row
    shared = {
        'bsh': (0.5 * bs[0]).astype(bf),
        'VsT': VsTh.astype(bf),
        'WT': np.ascontiguousarray(W.T).astype(bf),
        'Wpk': Wpk,
    }

    in_maps = []
    for core in range(8):
        b, h = core // 2, core % 2
        tmap = list(range(16)) if h == 0 else list(range(6, 16)) + list(range(6))
        Xp = X[b][:, :, tmap]                              # (N, F, 16)
        Xn = np.ascontiguousarray(Xp.transpose(0, 2, 1).reshape(N, T * F)).astype(bf)
        Xw = np.ascontiguousarray(Xp.transpose(2, 1, 0).reshape(8, P, N)).astype(bf)
        UW = np.zeros((8, P, 48), np.float32)
        for tp in range(16):
            s, v = tp // 2, tp % 2
            UW[s, 64 * v:64 * v + 64, tp] = Ws3
            UW[s, 64 * v:64 * v + 64, 32 + tp] = U3
        Pb = np.zeros((P, PBW), np.float32)
        Pb[:, 0:4] = U1.reshape(4, P).T
        Pb[:, 4:20] = np.vstack([Ws2, Ws2])
        VeTh = 0.5 * Ve[np.ix_(tmap, tmap)].T
        Pb[0:16, 20:36] = VeTh
        Pb[0:16, 36] = Ws1[tmap]
        Pb[0, 37:165] = 1.0
        Pb[:, 165:293] = np.eye(P, dtype=np.float32)
        Pb[0:64, 293:805] = U2
        Pb[0, 805:1061] = np.eye(T, dtype=np.float32).reshape(-1)
        # B2: (128, 2) block indicator * 1/64 for per-v mean over f
        Pb[0:64, 1061] = 1.0 / 64
        Pb[64:128, 1062] = 1.0 / 64
        # B2T: (2, 128) block indicator for broadcast back, gamma folded in
        # (replicated at partition 32 for the grouped-stats row-32 slices)
        Pb[0, 1063:1127] = gamma
        Pb[1, 1127:1191] = gamma
        Pb[32, 1063:1127] = gamma
        Pb[33, 1127:1191] = gamma
        # sigmoid-fold rank-1 rows
        Pb[0, 1191:1207] = VeTh.sum(axis=0)
        Pb[0, 1207:1719] = vch
        Pb[0:16, 1719:1735] = 0.5 * be[0][np.ix_(tmap, tmap)]
        Pfc = Pf.copy()
        Pfc[0:16, 132:148] = 0.5 * be[0][np.ix_(tmap, tmap)]
        m = dict(shared)
        m.update({
            'Xn': Xn, 'Xw': Xw, 'UW': UW.astype(bf),
            'Pb': Pb.astype(bf), 'Pf': Pfc,
        })
        in_maps.append(m)
    return in_maps


def kernel(**inputs):
    import sys
    if '/opt/trn_rl_repo' not in sys.path:
        sys.path.insert(0, '/opt/trn_rl_repo')
    from concourse.bass_utils import run_bass_kernel_spmd

    if 'nc' not in _CACHE:
        _CACHE['nc'] = _build_program()
    nc = _CACHE['nc']

    in_maps = _host_prep(inputs)
    res = run_bass_kernel_spmd(nc, in_maps, list(range(8)))
    out = np.zeros((B, N, F, T), np.float32)
    for core in range(8):
        b, h = core // 2, core % 2
        Z = np.asarray(res.results[core]['Zout']).astype(np.float32)
        # rows q*128 + v*64 + f, cols n  ->  (n, f, slot=2q+v)
        Zs = Z.reshape(NP, 2, F, N).transpose(3, 2, 0, 1).reshape(N, F, NSLOT)
        wstart = 0 if h == 0 else 6
        jlo = 0 if h == 0 else 2
        out[b, :, :, wstart + jlo:wstart + jlo + 8] = Zs[:, :, jlo:jlo + 8]
    return out


# revision 63
# speedup vs baseline: 1.0140x; 1.0140x over previous
"""ASTGCN block Trainium2 kernel (v2).

Strategy: 8 cores; core c handles batch b = c//2, time-half h = c%2 (8 output
timesteps each, data-parallel over B and T). Attention (temporal Et, spatial
S) is per-b and replicated on the 2 cores sharing a b. The sparse graph
propagation is reformulated as dense (N,N) matmuls: the edge-scatter of the
symmetric norm is accumulated host-side into a dense W (the +I/-I self-loop
terms cancel), so  prop1(h) = (W*S) @ h  and  prop2(h) = W @ h.

v2 changes vs baseline:
- Input DMAs ordered by first use (Pb/Pf/Xn first) and X tensors split in
  halves so attention matmuls start ~5us in instead of after all loads.
- Single activation-table regime: sigmoid via tanh (0.5*tanh(x/2)+0.5, in
  the exp table) and LN rstd via exp(-0.5*ln(var+eps)); only one table
  switch in the whole program (exp_and_others -> natural_log_exp...).
- LayerNorm runs in pair layout: per-pair stats via ones-block matmuls on
  PE (reduce over the f partition rows), rstd/-mu*rstd broadcast back with
  block matmuls; no transposes of the conv output at all.
- Output stored in pair layout as bf16; host does the final (f,n) -> (n,f)
  transpose and fp32 upcast.
- cheb -> conv -> LN -> store software-pipelined across the 5 timestep
  pairs to keep PE dense (p-state) and overlap store DMAs with compute.

Per-core time axis is PERMUTED so the program is identical SPMD: slot t' maps
to global t via tmap (identity for h=0, rotated by 6 for h=1); all
t-dependent weights (be, Ve, Ws1, UW) are permuted host-side to match.
"""

import numpy as np

B, N, F, T = 4, 512, 64, 16
P = 128
CH = N // P            # 4 n-chunks
NSLOT = 10             # cheb window timesteps per core (5 pairs)
NP = NSLOT // 2        # 5 pairs
LN_EPS = 1e-5

PBW = 1735             # packed bf16 constant width
PFW = 148              # packed f32 constant width

_CACHE = {}


def _build_program():
    import sys
    if '/opt/trn_rl_repo' not in sys.path:
        sys.path.insert(0, '/opt/trn_rl_repo')
    from contextlib import ExitStack
    import concourse.bass as bass
    import concourse.tile as tile
    from concourse import bacc, mybir

    dt = mybir.dt
    AL = mybir.AluOpType
    AF = mybir.ActivationFunctionType
    AX = mybir.AxisListType
    f32 = dt.float32
    bf16 = dt.bfloat16

    nc = bacc.Bacc("TRN2", target_bir_lowering=False, debug=False, num_devices=1)

    def din(name, shape, d=bf16):
        return nc.dram_tensor(name, list(shape), d, kind="ExternalInput").ap()

    XnD   = din("Xn", (N, T * F))
    XwD   = din("Xw", (8, P, N))
    UWD   = din("UW", (8, P, 48))
    bshD  = din("bsh", (N, N))          # 0.5 * bs
    VsTD  = din("VsT", (N, N))
    WTD   = din("WT", (N, N))
    WpkD  = din("Wpk", (7, P, P))
    PbD   = din("Pb", (P, PBW))
    PfD   = din("Pf", (P, PFW), f32)
    ZoutD = nc.dram_tensor("Zout", [NP * P, N], bf16, kind="ExternalOutput").ap()

    with tile.TileContext(nc) as tc, ExitStack() as ctx:
        sg = ctx.enter_context(tc.tile_pool(name="sg", bufs=1))
        big = ctx.enter_context(tc.tile_pool(name="big", bufs=5, space="PSUM"))
        sml = ctx.enter_context(tc.tile_pool(name="sml", bufs=2, space="PSUM"))
        hlf = ctx.enter_context(tc.tile_pool(name="hlf", bufs=1, space="PSUM"))
        xhp = ctx.enter_context(tc.tile_pool(name="xhp", bufs=7))
        txp = ctx.enter_context(tc.tile_pool(name="txp", bufs=5))
        lnp = ctx.enter_context(tc.tile_pool(name="lnp", bufs=5))

        # ------------- input DMAs, ordered by first use -------------
        Pb = sg.tile([P, PBW], bf16, tag="pb")
        nc.sync.dma_start(out=Pb[:], in_=PbD)
        Pf = sg.tile([P, PFW], f32, tag="pf")
        nc.sync.dma_start(out=Pf[:], in_=PfD)
        XnA = sg.tile([P, 2, T * F], bf16, tag="xna")
        XnB = sg.tile([P, 2, T * F], bf16, tag="xnb")
        XnDr = XnD.rearrange("(k p) t -> p k t", k=CH)
        UWAll = sg.tile([P, 8, 48], bf16, tag="uwall")
        XwA = sg.tile([P, 4, N], bf16, tag="xwa")
        XwB = sg.tile([P, 4, N], bf16, tag="xwb")
        XwDr = XwD.rearrange("s p n -> p s n")
        # interleave the two X layouts so both attention input paths
        # (lhs0 over Xn, R48 over Xw) can start on half the data
        nc.sync.dma_start(out=XnA[:], in_=XnDr[:, 0:2, :])
        nc.sync.dma_start(out=UWAll[:], in_=UWD.rearrange("s p n -> p s n"))
        nc.sync.dma_start(out=XwA[:], in_=XwDr[:, 0:4, :])
        nc.sync.dma_start(out=XnB[:], in_=XnDr[:, 2:4, :])
        nc.sync.dma_start(out=XwB[:], in_=XwDr[:, 4:8, :])
        bsAll = sg.tile([P, CH, N], bf16, tag="bsall")
        nc.sync.dma_start(out=bsAll[:], in_=bshD.rearrange("(k p) n -> p k n", k=CH))
        VsTAll = sg.tile([P, CH, N], bf16, tag="vstall")
        nc.sync.dma_start(out=VsTAll[:], in_=VsTD.rearrange("(k p) n -> p k n", k=CH))
        WTAll = sg.tile([P, CH, N], bf16, tag="wtall")
        nc.sync.dma_start(out=WTAll[:], in_=WTD.rearrange("(k p) n -> p k n", k=CH))
        Wpk = sg.tile([P, 7, P], bf16, tag="wpk")
        nc.sync.dma_start(out=Wpk[:], in_=WpkD.rearrange("w p c -> p w c"))

        Xn = [XnA[:, 0, :], XnA[:, 1, :], XnB[:, 0, :], XnB[:, 1, :]]
        Xw = [XwA[:, s, :] for s in range(4)] + [XwB[:, s, :] for s in range(4)]
        UW = [UWAll[:, s, :] for s in range(8)]
        bsh = [bsAll[:, k, :] for k in range(CH)]
        VsT = [VsTAll[:, k, :] for k in range(CH)]
        WT = [WTAll[:, k, :] for k in range(CH)]
        WcP = [Wpk[:, k, :] for k in range(3)]
        Lprev, Lmid, Lnext, WrP = (Wpk[:, 3, :], Wpk[:, 4, :], Wpk[:, 5, :],
                                   Wpk[:, 6, :])
        # packed bf16 layout
        U1r = Pb[:, 0:4]
        Ws2d = Pb[:, 4:20]
        VeT = Pb[0:16, 20:36]
        Ws1 = Pb[0:16, 36:37]
        ones1 = Pb[0:1, 37:165]
        I128b = Pb[:, 165:293]
        U2 = Pb[0:64, 293:805]
        I16r = Pb[0:1, 805:1061]     # I16 rows flattened: e_t = [0:1, 16t:16t+16]
        B2 = Pb[:, 1061:1063]        # (128,2) block col-indicator * 1/64
        B2T = Pb[0:2, 1063:1191]     # (2,128) block row-indicator * gamma[f]
        B2T32 = Pb[32:34, 1063:1191]  # same rows replicated at partition 32
        hcVe = Pb[0:1, 1191:1207]    # 0.5*colsum(VeT')  [sigmoid-fold row]
        vch = Pb[0:1, 1207:1719]     # 0.5*colsum(VsT')  [sigmoid-fold row]
        bePb = Pb[0:16, 1719:1735]   # 0.5*be (permuted), bf16
        # packed f32 layout
        gamP = Pf[:, 0:1]
        betP = Pf[:, 1:2]
        bch = Pf[:, 2:3]
        btr = Pf[:, 3:4]
        I128f = Pf[:, 4:132]
        bePh = Pf[0:16, 132:148]     # 0.5 * be (permuted)

        zerot = sg.tile([P, N], bf16, tag="zerot")
        nc.vector.memset(zerot[:], 0.0)
        epsP = sg.tile([P, 1], f32, tag="epsP")
        nc.vector.memset(epsP[:], LN_EPS)

        # persistent sbuf intermediates
        G = [sg.tile([P, N], bf16, tag=f"g{k}", name=f"g{k}") for k in range(CH)]
        Ex = [sg.tile([P, N], bf16, tag=f"ex{k}", name=f"ex{k}") for k in range(CH)]
        A1T = [sg.tile([P, N], bf16, tag=f"a1t{k}", name=f"a1t{k}") for k in range(CH)]
        dSv = [sg.tile([P, 1], f32, tag=f"dsv{k}", name=f"dsv{k}") for k in range(CH)]
        Tx0n = [sg.tile([P, T * F], bf16, tag=f"tx0n{k}", name=f"tx0n{k}")
                for k in range(CH)]
        dSB = sg.tile([P, N], bf16, tag="dsb")

        # =====================================================
        # Attention phase
        # =====================================================
        # ---- lhs0[(t,f)] = sum_n U1[n] X[n,(t,f)]  -> (1,1024)
        # accumulation interleaved with the R48 first half so PE follows the
        # XnA / XwA / XnB / XwB DMA arrival order
        L0a = sml.tile([1, 512], f32, tag="sml", name="l0a")
        L0b = sml.tile([1, 512], f32, tag="sml", name="l0b")
        R48p = hlf.tile([48, N], f32, tag="hlf", name="r48")
        for k in range(2):
            nc.tensor.matmul(L0a[:], U1r[:, k:k + 1], Xn[k][:, 0:512],
                             start=(k == 0), stop=False)
        for k in range(2):
            nc.tensor.matmul(L0b[:], U1r[:, k:k + 1], Xn[k][:, 512:1024],
                             start=(k == 0), stop=False)
        for s in range(4):
            nc.tensor.matmul(R48p[:], UW[s][:, :], Xw[s][:, :],
                             start=(s == 0), stop=False)
        for k in range(2, CH):
            nc.tensor.matmul(L0a[:], U1r[:, k:k + 1], Xn[k][:, 0:512],
                             start=False, stop=(k == CH - 1))
        for k in range(2, CH):
            nc.tensor.matmul(L0b[:], U1r[:, k:k + 1], Xn[k][:, 512:1024],
                             start=False, stop=(k == CH - 1))
        lhs0row = sg.tile([1, T * F], bf16, tag="lhs0row")
        nc.vector.tensor_copy(lhs0row[:, 0:512], L0a[:])
        nc.vector.tensor_copy(lhs0row[:, 512:1024], L0b[:])
        # reshape to (64,16) via 16 rank-1 matmuls against identity rows
        l0Fp = sml.tile([F, T], f32, tag="sml", name="l0fp")
        for t in range(T):
            nc.tensor.matmul(l0Fp[:], lhs0row[0:1, 64 * t:64 * t + 64],
                             I16r[0:1, 16 * t:16 * t + 16],
                             start=(t == 0), stop=(t == T - 1))
        # 0.5 sigmoid-prefactor folded here: scales lhs2T and hence P0
        lhs0F = sg.tile([F, T], bf16, tag="lhs0f")
        nc.vector.tensor_scalar(lhs0F[:], l0Fp[:], 0.5, None, op0=AL.mult)

        # ---- lhs2T chunks (n,16) = U2[:,chunk].T @ lhs0F, packed in one psum
        l2p = sml.tile([P, CH * T], f32, tag="sml", name="l2t")
        for k in range(CH):
            nc.tensor.matmul(l2p[:, k * T:(k + 1) * T],
                             U2[:, k * P:(k + 1) * P], lhs0F[:],
                             start=True, stop=True)
        l2s = sg.tile([P, CH * T], bf16, tag="l2ts")
        nc.vector.tensor_copy(l2s[:], l2p[:])
        lhs2T = [l2s[:, k * T:(k + 1) * T] for k in range(CH)]

        # ---- R48 second half: rows 0:16 rhs3T (Ws3), rows 32:48 rhs_tT (U3)
        for s in range(4, 8):
            nc.tensor.matmul(R48p[:], UW[s][:, :], Xw[s][:, :],
                             start=False, stop=(s == 7))
        R48 = sg.tile([48, N], bf16, tag="r48s")
        nc.scalar.copy(R48[:], R48p[:])

        # ---- rhs_tn chunks: transpose R48[32:48], packed in one psum
        rtp = hlf.tile([P, CH * T], bf16, tag="hlf", name="rtn")
        for k in range(CH):
            nc.tensor.transpose(rtp[:, k * T:(k + 1) * T],
                                R48[32:48, k * P:(k + 1) * P],
                                I128b[32:48, 32:48])
        rts = sg.tile([P, CH * T], bf16, tag="rtns")
        nc.vector.tensor_copy(rts[:], rtp[:])
        rhs_tn = [rts[:, k * T:(k + 1) * T] for k in range(CH)]

        # ---- P0 (16,16) = 0.5*(lhs_t @ rhs_t) + 0.5*be (bias via I16 matmul)
        P0p = sml.tile([T, T], f32, tag="sml", name="p0")
        for k in range(CH):
            nc.tensor.matmul(P0p[:], lhs2T[k][:], rhs_tn[k][:],
                             start=(k == 0), stop=False)
        nc.tensor.matmul(P0p[:], I128b[0:16, 0:16], bePb[:],
                         start=False, stop=True)
        # sig holds tanh(0.5*(P0+be)); sigmoid affine folded into E1T matmul
        # (VeT is 0.5-scaled host-side, hcVe rank-1 term)
        sig = sg.tile([T, T], bf16, tag="sig")
        nc.scalar.activation(sig[:], P0p[:], AF.Tanh)

        # ---- E1^T = sigmoid^T @ Ve^T ; softmax over free dim
        E1Tp = sml.tile([T, T], f32, tag="sml", name="e1t")
        nc.tensor.matmul(E1Tp[:], sig[:], VeT[:], start=True, stop=False)
        nc.tensor.matmul(E1Tp[:], ones1[0:1, 0:16], hcVe[:],
                         start=False, stop=True)
        # values are O(1e-1): skip the max-subtraction for softmax
        sume = sg.tile([T, 1], f32, tag="sume")
        EtT = sg.tile([T, T], bf16, tag="ett")
        nc.scalar.activation(EtT[:], E1Tp[:], AF.Exp,
                             scale=1.0, accum_out=sume[:, 0:1])
        rse = sg.tile([T, 1], f32, tag="rse")
        nc.vector.reciprocal(rse[:], sume[:])
        nc.vector.tensor_scalar(EtT[:], EtT[:], rse[:, 0:1], None, op0=AL.mult)
        Etp = hlf.tile([T, T], bf16, tag="hlf", name="etp")
        nc.tensor.transpose(Etp[:], EtT[:], I128b[0:16, 0:16])
        Et = sg.tile([T, T], bf16, tag="et")
        nc.vector.tensor_copy(Et[:], Etp[:])

        # ---- w1e row (1,16) = Ws1.T @ EtT ; broadcast straight into the
        # pair layout w1Bpair[p=(v,f), s] = w1e[2s+v] with two base-64 matmuls
        w1p = sml.tile([1, T], f32, tag="sml", name="w1p")
        nc.tensor.matmul(w1p[:], Ws1[:], EtT[:], start=True, stop=True)
        w1row = sg.tile([1, T], bf16, tag="w1row")
        nc.scalar.copy(w1row[:], w1p[:])
        w1B2p = sml.tile([P, 8], f32, tag="sml", name="w1b2p")
        nc.tensor.matmul(w1B2p[0:64, :], ones1[0:1, 0:64],
                         w1row[:, 0:T:2], start=True, stop=True)
        nc.tensor.matmul(w1B2p[64:128, :], ones1[0:1, 0:64],
                         w1row[:, 1:T:2], start=True, stop=True)
        # ---- Ws2wP[p=(v,f), s, t] = Ws2d[p,t] * w1e[2s+v]   (128, 8, 16)
        # (reads the w1e broadcast straight from PSUM, saves a copy hop)
        Ws2w = sg.tile([P, 8, T], bf16, tag="ws2w")
        nc.vector.tensor_tensor(
            Ws2w[:],
            Ws2d[:].unsqueeze(1).broadcast_to((P, 8, T)),
            w1B2p[:].unsqueeze(2).broadcast_to((P, 8, T)),
            op=AL.mult)

        # ---- lhs_sT (16, 512) = sum_t1 (Ws2*w1e[t1]).T @ X^T[t1]
        lsTp = sml.tile([T, N], f32, tag="sml", name="lst")
        for s in range(8):
            nc.tensor.matmul(lsTp[:], Ws2w[:, s, :], Xw[s][:, :],
                             start=(s == 0), stop=(s == 7))
        lsT = sg.tile([T, N], bf16, tag="lsts")
        nc.scalar.copy(lsT[:], lsTp[:])

        # ---- rhs_s (16, 512) = Et-weighted rhs3; 0.5 sigmoid-prefactor folded
        rsp = sml.tile([T, N], f32, tag="sml", name="rsp")
        nc.tensor.matmul(rsp[:], Et[:], R48[0:16, :], start=True, stop=True)
        rss = sg.tile([T, N], bf16, tag="rss")
        nc.scalar.mul(rss[:], rsp[:], 0.5)

        # ---- P chunks; bs bias absorbed via identity matmul; G holds
        # tanh(0.5*P + bsh); the sigmoid affine is folded into M1T
        # (VsT 0.5-scaled host-side + vch rank-1 term)
        for k in range(CH):
            Pp = big.tile([P, N], f32, tag="big", name="pp")
            nc.tensor.matmul(Pp[:], lsT[:, k * P:(k + 1) * P], rss[:],
                             start=True, stop=False)
            nc.tensor.matmul(Pp[:], I128b[:], bsh[k][:],
                             start=False, stop=True)
            nc.scalar.activation(G[k][:], Pp[:], AF.Tanh)

        # ---- M1T chunks (c-part, r) + masked softmax -> A1T, dS
        for c in range(CH):
            Mp = big.tile([P, N], f32, tag="big", name="mp")
            for k in range(CH):
                nc.tensor.matmul(Mp[:], G[k][:, c * P:(c + 1) * P], VsT[k][:],
                                 start=(k == 0), stop=False)
            nc.tensor.matmul(Mp[:], ones1[0:1, 0:128], vch[:],
                             start=False, stop=True)
            sme = sg.tile([P, 1], f32, tag=f"sme{c}", name=f"sme{c}")
            nc.scalar.activation(Ex[c][:], Mp[:], AF.Exp,
                                 scale=1.0, accum_out=sme[:, 0:1])
            rcp = sg.tile([P, 1], f32, tag=f"rcp{c}", name=f"rcp{c}")
            nc.vector.reciprocal(rcp[:], sme[:])
            # A1T = (Ex * rcp) * WT   (= S^T o W^T)
            nc.vector.scalar_tensor_tensor(A1T[c][:], Ex[c][:], rcp[:, 0:1],
                                           WT[c][:], op0=AL.mult, op1=AL.mult)
            # diag: dS = sum_r (Ex*rcp)*I over the diagonal block
            dtmp = sg.tile([P, P], bf16, tag="dtmp")
            nc.vector.scalar_tensor_tensor(dtmp[:], Ex[c][:, c * P:(c + 1) * P],
                                           rcp[:, 0:1], I128b[:],
                                           op0=AL.mult, op1=AL.mult)
            nc.vector.tensor_reduce(dSv[c][:], dtmp[:], axis=AX.X, op=AL.add)

        # ---- dS row + broadcast tile (128, 512)
        dSrp = sml.tile([1, N], f32, tag="sml", name="dsrp")
        for c in range(CH):
            nc.tensor.transpose(dSrp[:, c * P:(c + 1) * P], dSv[c][:], I128f[:])
        dSrow = sg.tile([1, N], bf16, tag="dsrow")
        nc.scalar.copy(dSrow[:], dSrp[:])
        dSBp = sml.tile([P, N], f32, tag="sml", name="dsbp")
        nc.tensor.matmul(dSBp[:], ones1[:], dSrow[:], start=True, stop=True)
        nc.scalar.copy(dSB[:], dSBp[:])

        # ---- Tx0 in n-layout (all t at once)
        for k in range(CH):
            nc.vector.tensor_scalar(Tx0n[k][:], Xn[k][:], dSv[k][:, 0:1], None,
                                    op0=AL.mult)

        # =====================================================
        # Cheb + conv + LN, software-pipelined per pair
        # =====================================================
        Tx0P = {}
        TAp = {}
        Tx1T = {}
        ptA = {}
        Tx1n = {}
        TBp = {}
        Tx2T = {}
        TCp = {}
        XhP = {-1: zerot, NP: zerot}
        TDp = {}
        ZT = {}

        def e_tx0p(q):
            t = txp.tile([P, N], bf16, tag="tx0p", name=f"tx0p{q}")
            nc.gpsimd.tensor_tensor(t[:], Xw[q][:], dSB[:], op=AL.mult)
            Tx0P[q] = t

        def e_ta(q):
            p = big.tile([P, N], f32, tag="big", name=f"ta{q}")
            for k in range(CH):
                lhs = Tx0n[k][:, 2 * q * F:(2 * q + 2) * F]
                nc.tensor.matmul(p[:], lhs, A1T[k][:],
                                 start=(k == 0), stop=(k == CH - 1))
            TAp[q] = p

        def e_b(q):
            t = txp.tile([P, N], bf16, tag="tx1t", name=f"tx1t{q}")
            nc.vector.tensor_copy(t[:], TAp[q][:])
            Tx1T[q] = t

        def e_c(q):
            p = hlf.tile([P, N], bf16, tag="hlf", name=f"pta{q}")
            for k in range(CH):
                nc.tensor.transpose(p[:, k * P:(k + 1) * P],
                                    Tx1T[q][:, k * P:(k + 1) * P], I128b[:])
            ptA[q] = p

        def e_d(q):
            t = txp.tile([P, N], bf16, tag="tx1n", name=f"tx1n{q}")
            nc.scalar.copy(t[:], ptA[q][:])
            Tx1n[q] = t

        def e_e(q):
            p = big.tile([P, N], f32, tag="big", name=f"tb{q}")
            for k in range(CH):
                nc.tensor.matmul(p[:], Tx1n[q][:, k * P:(k + 1) * P], WT[k][:],
                                 start=(k == 0), stop=(k == CH - 1))
            TBp[q] = p

        def e_f(q):
            t = txp.tile([P, N], bf16, tag="tx2t", name=f"tx2t{q}")
            nc.vector.scalar_tensor_tensor(t[:], TBp[q][:], 2.0, Tx0P[q][:],
                                           op0=AL.mult, op1=AL.subtract)
            Tx2T[q] = t

        def e_g(q):
            p = big.tile([P, N], f32, tag="big", name=f"tc{q}")
            nc.tensor.matmul(p[:], WcP[0][:], Tx0P[q][:], start=True, stop=False)
            nc.tensor.matmul(p[:], WcP[1][:], Tx1T[q][:], start=False, stop=False)
            nc.tensor.matmul(p[:], WcP[2][:], Tx2T[q][:], start=False, stop=True)
            TCp[q] = p

        def e_h(q):
            t = xhp.tile([P, N], bf16, tag="xh", name=f"xh{q}")
            nc.scalar.activation(t[:], TCp[q][:], AF.Relu, bias=bch[:, 0:1],
                                 scale=1.0)
            XhP[q] = t

        def e_i(q):
            p = big.tile([P, N], f32, tag="big", name=f"td{q}")
            nc.tensor.matmul(p[:], Lprev[:], XhP[q - 1][:], start=True, stop=False)
            nc.tensor.matmul(p[:], Lmid[:], XhP[q][:], start=False, stop=False)
            nc.tensor.matmul(p[:], Lnext[:], XhP[q + 1][:], start=False, stop=False)
            nc.tensor.matmul(p[:], WrP[:], Xw[q][:], start=False, stop=True)
            TDp[q] = p

        def e_j(q):
            t = lnp.tile([P, N], bf16, tag="zt", name=f"zt{q}")
            nc.scalar.activation(t[:], TDp[q][:], AF.Relu, bias=btr[:, 0:1],
                                 scale=1.0)
            ZT[q] = t

        RSTD = {}
        NMR = {}
        SQ = {}
        BASE32 = {}

        def e_sq(q):
            t = lnp.tile([P, N], bf16, tag="sq", name=f"sq{q}")
            nc.vector.tensor_tensor(t[:], ZT[q][:], ZT[q][:], op=AL.mult)
            SQ[q] = t

        def e_ln1(qa, qb):
            # batched stats for a PAIR GROUP: pair qa in rows 0:2, pair qb in
            # rows 32:34 (matmul out base partition must be 0/32/64). One
            # small-op chain then serves two timestep pairs; rows 2:32 are
            # never-read garbage.
            rows = 34 if qb is not None else 2
            sA = sml.tile([34, N], f32, tag="sml", name=f"sA{qa}")
            sB = sml.tile([34, N], f32, tag="sml", name=f"sB{qa}")
            nc.tensor.matmul(sA[0:2, :], B2[:], ZT[qa][:], start=True, stop=True)
            nc.tensor.matmul(sB[0:2, :], B2[:], SQ[qa][:], start=True, stop=True)
            if qb is not None:
                nc.tensor.matmul(sA[32:34, :], B2[:], ZT[qb][:],
                                 start=True, stop=True)
                nc.tensor.matmul(sB[32:34, :], B2[:], SQ[qb][:],
                                 start=True, stop=True)
            mu = lnp.tile([34, N], f32, tag="mu", name=f"mu{qa}")
            nc.scalar.copy(mu[0:rows, :], sA[0:rows, :])
            mu2 = lnp.tile([34, N], f32, tag="mu2", name=f"mu2{qa}")
            # Pool while the pipe is full (throughput), DVE in the drain
            # (latency: Pool elementwise runs at 0.42 efficiency)
            eng = nc.gpsimd if qa == 0 else nc.vector
            eng.tensor_tensor(mu2[0:rows, :], mu[0:rows, :], mu[0:rows, :],
                              op=AL.mult)
            # var+eps = (msq + eps) - mu^2 in one stt
            var = lnp.tile([34, N], f32, tag="var", name=f"var{qa}")
            nc.vector.scalar_tensor_tensor(var[0:rows, :], sB[0:rows, :],
                                           LN_EPS, mu2[0:rows, :],
                                           op0=AL.add, op1=AL.subtract)
            # rstd = sqrt(1/(var+eps)); approx recip is ~18 bits, plenty
            rv = lnp.tile([34, N], f32, tag="rv", name=f"rv{qa}")
            nc.vector.reciprocal_approx_fast(rv[0:rows, :], var[0:rows, :])
            rstd = lnp.tile([34, N], bf16, tag="rstd", name=f"rstd{qa}")
            nc.scalar.activation(rstd[0:rows, :], rv[0:rows, :], AF.Sqrt)
            # nmr = mu * rstd
            nmr = lnp.tile([34, N], bf16, tag="nmr", name=f"nmr{qa}")
            eng.tensor_tensor(nmr[0:rows, :], mu[0:rows, :], rstd[0:rows, :],
                              op=AL.mult)
            RSTD[qa] = rstd[0:2, :]
            NMR[qa] = nmr[0:2, :]
            BASE32[qa] = False
            if qb is not None:
                RSTD[qb] = rstd[32:34, :]
                NMR[qb] = nmr[32:34, :]
                BASE32[qb] = True

        def e_ln2(q):
            z = ZT[q]
            # broadcast to 128 partitions; gamma[f] is folded into B2T
            bt = B2T32 if BASE32[q] else B2T
            rBp = big.tile([P, N], f32, tag="big", name=f"rbp{q}")
            nc.tensor.matmul(rBp[:], bt[:], RSTD[q][:], start=True, stop=True)
            nBp = big.tile([P, N], f32, tag="big", name=f"nbp{q}")
            nc.tensor.matmul(nBp[:], bt[:], NMR[q][:], start=True, stop=True)
            # w = z*(rstd*gam)B + bet - (mu*rstd*gam)B
            u = lnp.tile([P, N], bf16, tag="u", name=f"u{q}")
            nc.vector.tensor_tensor(u[:], z[:], rBp[:], op=AL.mult)
            w = lnp.tile([P, N], bf16, tag="w", name=f"w{q}")
            nc.vector.scalar_tensor_tensor(w[:], u[:], betP[:, 0:1], nBp[:],
                                           op0=AL.add, op1=AL.subtract)
            nc.sync.dma_start(out=ZoutD[q * P:(q + 1) * P, :], in_=w[:])

        # pipeline drive, depth 7: every PE group's inputs are produced in a
        # PREVIOUS iteration, so the in-order PE queue never head-of-line
        # blocks. Stage distances: TA@0, trans@1, TB@2, TC@3, TD@4, ln1@5,
        # ln2@7. Cross-engine hops (stt, relu, copies) happen within the
        # iteration that produced their psum input.
        def live(q):
            return 0 <= q < NP

        for i in range(NP + 7):
            if live(i - 7):
                e_ln2(i - 7)
            if live(i - 1):
                e_c(i - 1)
            if live(i):
                if i == 0:
                    e_tx0p(0)
                e_ta(i)
            if live(i - 1):
                e_d(i - 1)
            if live(i - 2):
                e_e(i - 2)
                e_f(i - 2)
            if live(i - 3):
                e_g(i - 3)
                e_h(i - 3)
            if live(i):
                e_b(i)
                if live(i + 1):
                    e_tx0p(i + 1)
            if live(i - 4):
                e_i(i - 4)
                e_j(i - 4)
                e_sq(i - 4)
            # grouped stats chains: 0 @ i=5, (1,2) @ i=7, (3,4) @ i=9
            if i == 5:
                e_ln1(0, None)
            elif i == 7:
                e_ln1(1, 2)
            elif i == 9:
                e_ln1(3, 4)

    nc.compile()
    return nc


def _host_prep(inputs):
    import ml_dtypes
    bf = ml_dtypes.bfloat16

    X = np.asarray(inputs['X'], np.float32)
    edge_index = np.asarray(inputs['edge_index'])
    U1 = np.asarray(inputs['U1'], np.float32)
    U2 = np.asarray(inputs['U2'], np.float32)
    U3 = np.asarray(inputs['U3'], np.float32)
    be = np.asarray(inputs['be'], np.float32)
    Ve = np.asarray(inputs['Ve'], np.float32)
    Ws1 = np.asarray(inputs['Ws1'], np.float32)
    Ws2 = np.asarray(inputs['Ws2'], np.float32)
    Ws3 = np.asarray(inputs['Ws3'], np.float32)
    bs = np.asarray(inputs['bs'], np.float32)
    Vs = np.asarray(inputs['Vs'], np.float32)
    W_cheb = np.asarray(inputs['W_cheb'], np.float32)
    b_cheb = np.asarray(inputs['b_cheb'], np.float32)
    Wt = np.asarray(inputs['Wt'], np.float32)
    bt = np.asarray(inputs['bt'], np.float32)
    Wr = np.asarray(inputs['Wr'], np.float32)
    br = np.asarray(inputs['br'], np.float32)
    gamma = np.asarray(inputs['gamma'], np.float32)
    beta = np.asarray(inputs['beta'], np.float32)

    # dense symmetric-norm matrix (self-loop +I/-I terms cancel)
    row, col = edge_index[0].astype(np.int64), edge_index[1].astype(np.int64)
    deg = np.zeros(N, np.float32)
    np.add.at(deg, row, 1.0)
    dis = np.where(deg > 0, 1.0 / np.sqrt(np.maximum(deg, 1.0)), 0.0).astype(np.float32)
    wn = -dis[row] * dis[col]
    W = np.zeros((N, N), np.float32)
    np.add.at(W, (row, col), wn)

    # conv block matrices: L[(v,fi),(u,fo)] = Wt[fo,fi,0,dt]
    WtT = [np.ascontiguousarray(Wt[:, :, 0, d].T) for d in range(3)]  # (fi,fo)
    Z64 = np.zeros((F, F), np.float32)
    Lmid = np.block([[WtT[1], WtT[0]], [WtT[2], WtT[1]]]).astype(bf)
    Lprev = np.block([[Z64, Z64], [WtT[0], Z64]]).astype(bf)
    Lnext = np.block([[Z64, WtT[2]], [Z64, Z64]]).astype(bf)
    WrT = np.ascontiguousarray(Wr[:, :, 0, 0].T)
    WrP = np.block([[WrT, Z64], [Z64, WrT]]).astype(bf)
    WcP = np.stack([np.block([[W_cheb[k], Z64], [Z64, W_cheb[k]]]) for k in range(3)]
                   ).astype(bf)

    Wpk = np.stack([WcP[0], WcP[1], WcP[2], Lprev, Lmid, Lnext, WrP])

    Pf = np.zeros((P, PFW), np.float32)
    Pf[:, 0] = np.tile(gamma, 2)
    Pf[:, 1] = np.tile(beta, 2)
    Pf[:, 2] = np.tile(b_cheb, 2)
    Pf[:, 3] = np.tile(bt + br, 2)
    Pf[:, 4:132] = np.eye(P, dtype=np.float32)

    VsTh = 0.5 * np.ascontiguousarray(Vs.T)
    vch = VsTh.sum(axis=0)                 # 0.5*colsum(Vs^T) sigmoid-fold row
    shared = {
        'bsh': (0.5 * bs[0]).astype(bf),
        'VsT': VsTh.astype(bf),
        'WT': np.ascontiguousarray(W.T).astype(bf),
        'Wpk': Wpk,
    }

    in_maps = []
    for core in range(8):
        b, h = core // 2, core % 2
        tmap = list(range(16)) if h == 0 else list(range(6, 16)) + list(range(6))
        Xp = X[b][:, :, tmap]                              # (N, F, 16)
        Xn = np.ascontiguousarray(Xp.transpose(0, 2, 1).reshape(N, T * F)).astype(bf)
        Xw = np.ascontiguousarray(Xp.transpose(2, 1, 0).reshape(8, P, N)).astype(bf)
        UW = np.zeros((8, P, 48), np.float32)
        for tp in range(16):
            s, v = tp // 2, tp % 2
            UW[s, 64 * v:64 * v + 64, tp] = Ws3
            UW[s, 64 * v:64 * v + 64, 32 + tp] = U3
        Pb = np.zeros((P, PBW), np.float32)
        Pb[:, 0:4] = U1.reshape(4, P).T
        Pb[:, 4:20] = np.vstack([Ws2, Ws2])
        VeTh = 0.5 * Ve[np.ix_(tmap, tmap)].T
        Pb[0:16, 20:36] = VeTh
        Pb[0:16, 36] = Ws1[tmap]
        Pb[0, 37:165] = 1.0
        Pb[:, 165:293] = np.eye(P, dtype=np.float32)
        Pb[0:64, 293:805] = U2
        Pb[0, 805:1061] = np.eye(T, dtype=np.float32).reshape(-1)
        # B2: (128, 2) block indicator * 1/64 for per-v mean over f
        Pb[0:64, 1061] = 1.0 / 64
        Pb[64:128, 1062] = 1.0 / 64
        # B2T: (2, 128) block indicator for broadcast back, gamma folded in
        # (replicated at partition 32 for the grouped-stats row-32 slices)
        Pb[0, 1063:1127] = gamma
        Pb[1, 1127:1191] = gamma
        Pb[32, 1063:1127] = gamma
        Pb[33, 1127:1191] = gamma
        # sigmoid-fold rank-1 rows
        Pb[0, 1191:1207] = VeTh.sum(axis=0)
        Pb[0, 1207:1719] = vch
        Pb[0:16, 1719:1735] = 0.5 * be[0][np.ix_(tmap, tmap)]
        Pfc = Pf.copy()
        Pfc[0:16, 132:148] = 0.5 * be[0][np.ix_(tmap, tmap)]
        m = dict(shared)
        m.update({
            'Xn': Xn, 'Xw': Xw, 'UW': UW.astype(bf),
            'Pb': Pb.astype(bf), 'Pf': Pfc,
        })
        in_maps.append(m)
    return in_maps


def kernel(**inputs):
    import sys
    if '/opt/trn_rl_repo' not in sys.path:
        sys.path.insert(0, '/opt/trn_rl_repo')
    from concourse.bass_utils import run_bass_kernel_spmd

    if 'nc' not in _CACHE:
        _CACHE['nc'] = _build_program()
    nc = _CACHE['nc']

    in_maps = _host_prep(inputs)
    res = run_bass_kernel_spmd(nc, in_maps, list(range(8)))
    out = np.zeros((B, N, F, T), np.float32)
    for core in range(8):
        b, h = core // 2, core % 2
        Z = np.asarray(res.results[core]['Zout']).astype(np.float32)
        # rows q*128 + v*64 + f, cols n  ->  (n, f, slot=2q+v)
        Zs = Z.reshape(NP, 2, F, N).transpose(3, 2, 0, 1).reshape(N, F, NSLOT)
        wstart = 0 if h == 0 else 6
        jlo = 0 if h == 0 else 2
        out[b, :, :, wstart + jlo:wstart + jlo + 8] = Zs[:, :, jlo:jlo + 8]
    return out


# revision 64
# speedup vs baseline: 1.0317x; 1.0174x over previous
"""ASTGCN block Trainium2 kernel (v2).

Strategy: 8 cores; core c handles batch b = c//2, time-half h = c%2 (8 output
timesteps each, data-parallel over B and T). Attention (temporal Et, spatial
S) is per-b and replicated on the 2 cores sharing a b. The sparse graph
propagation is reformulated as dense (N,N) matmuls: the edge-scatter of the
symmetric norm is accumulated host-side into a dense W (the +I/-I self-loop
terms cancel), so  prop1(h) = (W*S) @ h  and  prop2(h) = W @ h.

v2 changes vs baseline:
- Input DMAs ordered by first use (Pb/Pf/Xn first) and X tensors split in
  halves so attention matmuls start ~5us in instead of after all loads.
- Single activation-table regime: sigmoid via tanh (0.5*tanh(x/2)+0.5, in
  the exp table) and LN rstd via exp(-0.5*ln(var+eps)); only one table
  switch in the whole program (exp_and_others -> natural_log_exp...).
- LayerNorm runs in pair layout: per-pair stats via ones-block matmuls on
  PE (reduce over the f partition rows), rstd/-mu*rstd broadcast back with
  block matmuls; no transposes of the conv output at all.
- Output stored in pair layout as bf16; host does the final (f,n) -> (n,f)
  transpose and fp32 upcast.
- cheb -> conv -> LN -> store software-pipelined across the 5 timestep
  pairs to keep PE dense (p-state) and overlap store DMAs with compute.

Per-core time axis is PERMUTED so the program is identical SPMD: slot t' maps
to global t via tmap (identity for h=0, rotated by 6 for h=1); all
t-dependent weights (be, Ve, Ws1, UW) are permuted host-side to match.
"""

import numpy as np

B, N, F, T = 4, 512, 64, 16
P = 128
CH = N // P            # 4 n-chunks
NSLOT = 10             # cheb window timesteps per core (5 pairs)
NP = NSLOT // 2        # 5 pairs
LN_EPS = 1e-5

PBW = 1735             # packed bf16 constant width
PFW = 148              # packed f32 constant width

_CACHE = {}


def _build_program():
    import sys
    if '/opt/trn_rl_repo' not in sys.path:
        sys.path.insert(0, '/opt/trn_rl_repo')
    from contextlib import ExitStack
    import concourse.bass as bass
    import concourse.tile as tile
    from concourse import bacc, mybir

    dt = mybir.dt
    AL = mybir.AluOpType
    AF = mybir.ActivationFunctionType
    AX = mybir.AxisListType
    f32 = dt.float32
    bf16 = dt.bfloat16

    nc = bacc.Bacc("TRN2", target_bir_lowering=False, debug=False, num_devices=1)

    def din(name, shape, d=bf16):
        return nc.dram_tensor(name, list(shape), d, kind="ExternalInput").ap()

    XnD   = din("Xn", (N, T * F))
    XwD   = din("Xw", (8, P, N))
    UWD   = din("UW", (8, P, 48))
    bshD  = din("bsh", (N, N))          # 0.5 * bs
    VsTD  = din("VsT", (N, N))
    WTD   = din("WT", (N, N))
    WpkD  = din("Wpk", (7, P, P))
    PbD   = din("Pb", (P, PBW))
    PfD   = din("Pf", (P, PFW), f32)
    ZoutD = nc.dram_tensor("Zout", [NP * P, N], bf16, kind="ExternalOutput").ap()

    with tile.TileContext(nc) as tc, ExitStack() as ctx:
        sg = ctx.enter_context(tc.tile_pool(name="sg", bufs=1))
        big = ctx.enter_context(tc.tile_pool(name="big", bufs=5, space="PSUM"))
        sml = ctx.enter_context(tc.tile_pool(name="sml", bufs=2, space="PSUM"))
        hlf = ctx.enter_context(tc.tile_pool(name="hlf", bufs=1, space="PSUM"))
        xhp = ctx.enter_context(tc.tile_pool(name="xhp", bufs=7))
        txp = ctx.enter_context(tc.tile_pool(name="txp", bufs=5))
        lnp = ctx.enter_context(tc.tile_pool(name="lnp", bufs=5))

        # ------------- input DMAs, ordered by first use -------------
        Pb = sg.tile([P, PBW], bf16, tag="pb")
        nc.sync.dma_start(out=Pb[:], in_=PbD)
        Pf = sg.tile([P, PFW], f32, tag="pf")
        nc.sync.dma_start(out=Pf[:], in_=PfD)
        XnA = sg.tile([P, 2, T * F], bf16, tag="xna")
        XnB = sg.tile([P, 2, T * F], bf16, tag="xnb")
        XnDr = XnD.rearrange("(k p) t -> p k t", k=CH)
        UWAll = sg.tile([P, 8, 48], bf16, tag="uwall")
        XwA = sg.tile([P, 4, N], bf16, tag="xwa")
        XwB = sg.tile([P, 4, N], bf16, tag="xwb")
        XwDr = XwD.rearrange("s p n -> p s n")
        # interleave the two X layouts so both attention input paths
        # (lhs0 over Xn, R48 over Xw) can start on half the data
        nc.sync.dma_start(out=XnA[:], in_=XnDr[:, 0:2, :])
        nc.sync.dma_start(out=UWAll[:], in_=UWD.rearrange("s p n -> p s n"))
        nc.sync.dma_start(out=XwA[:], in_=XwDr[:, 0:4, :])
        nc.sync.dma_start(out=XnB[:], in_=XnDr[:, 2:4, :])
        nc.sync.dma_start(out=XwB[:], in_=XwDr[:, 4:8, :])
        bsAll = sg.tile([P, CH, N], bf16, tag="bsall")
        nc.sync.dma_start(out=bsAll[:], in_=bshD.rearrange("(k p) n -> p k n", k=CH))
        VsTAll = sg.tile([P, CH, N], bf16, tag="vstall")
        nc.sync.dma_start(out=VsTAll[:], in_=VsTD.rearrange("(k p) n -> p k n", k=CH))
        WTAll = sg.tile([P, CH, N], bf16, tag="wtall")
        nc.sync.dma_start(out=WTAll[:], in_=WTD.rearrange("(k p) n -> p k n", k=CH))
        Wpk = sg.tile([P, 7, P], bf16, tag="wpk")
        nc.sync.dma_start(out=Wpk[:], in_=WpkD.rearrange("w p c -> p w c"))

        Xn = [XnA[:, 0, :], XnA[:, 1, :], XnB[:, 0, :], XnB[:, 1, :]]
        Xw = [XwA[:, s, :] for s in range(4)] + [XwB[:, s, :] for s in range(4)]
        UW = [UWAll[:, s, :] for s in range(8)]
        bsh = [bsAll[:, k, :] for k in range(CH)]
        VsT = [VsTAll[:, k, :] for k in range(CH)]
        WT = [WTAll[:, k, :] for k in range(CH)]
        WcP = [Wpk[:, k, :] for k in range(3)]
        Lprev, Lmid, Lnext, WrP = (Wpk[:, 3, :], Wpk[:, 4, :], Wpk[:, 5, :],
                                   Wpk[:, 6, :])
        # packed bf16 layout
        U1r = Pb[:, 0:4]
        Ws2d = Pb[:, 4:20]
        VeT = Pb[0:16, 20:36]
        Ws1 = Pb[0:16, 36:37]
        ones1 = Pb[0:1, 37:165]
        I128b = Pb[:, 165:293]
        U2 = Pb[0:64, 293:805]
        I16r = Pb[0:1, 805:1061]     # I16 rows flattened: e_t = [0:1, 16t:16t+16]
        B2 = Pb[:, 1061:1063]        # (128,2) block col-indicator * 1/64
        B2T = Pb[0:2, 1063:1191]     # (2,128) block row-indicator * gamma[f]
        B2T32 = Pb[32:34, 1063:1191]  # same rows replicated at partition 32
        hcVe = Pb[0:1, 1191:1207]    # 0.5*colsum(VeT')  [sigmoid-fold row]
        vch = Pb[0:1, 1207:1719]     # 0.5*colsum(VsT')  [sigmoid-fold row]
        bePb = Pb[0:16, 1719:1735]   # 0.5*be (permuted), bf16
        # packed f32 layout
        gamP = Pf[:, 0:1]
        betP = Pf[:, 1:2]
        bch = Pf[:, 2:3]
        btr = Pf[:, 3:4]
        I128f = Pf[:, 4:132]
        bePh = Pf[0:16, 132:148]     # 0.5 * be (permuted)

        zerot = sg.tile([P, N], bf16, tag="zerot")
        nc.vector.memset(zerot[:], 0.0)
        epsP = sg.tile([P, 1], f32, tag="epsP")
        nc.vector.memset(epsP[:], LN_EPS)

        # persistent sbuf intermediates
        G = [sg.tile([P, N], bf16, tag=f"g{k}", name=f"g{k}") for k in range(CH)]
        Ex = [sg.tile([P, N], bf16, tag=f"ex{k}", name=f"ex{k}") for k in range(CH)]
        A1T = [sg.tile([P, N], bf16, tag=f"a1t{k}", name=f"a1t{k}") for k in range(CH)]
        dSv = [sg.tile([P, 1], f32, tag=f"dsv{k}", name=f"dsv{k}") for k in range(CH)]
        Tx0n = [sg.tile([P, T * F], bf16, tag=f"tx0n{k}", name=f"tx0n{k}")
                for k in range(CH)]
        dSB = sg.tile([P, N], bf16, tag="dsb")

        # =====================================================
        # Attention phase
        # =====================================================
        # ---- lhs0[(t,f)] = sum_n U1[n] X[n,(t,f)]  -> (1,1024)
        # accumulation interleaved with the R48 first half so PE follows the
        # XnA / XwA / XnB / XwB DMA arrival order
        L0a = sml.tile([1, 512], f32, tag="sml", name="l0a")
        L0b = sml.tile([1, 512], f32, tag="sml", name="l0b")
        R48p = hlf.tile([48, N], f32, tag="hlf", name="r48")
        for k in range(2):
            nc.tensor.matmul(L0a[:], U1r[:, k:k + 1], Xn[k][:, 0:512],
                             start=(k == 0), stop=False)
        for k in range(2):
            nc.tensor.matmul(L0b[:], U1r[:, k:k + 1], Xn[k][:, 512:1024],
                             start=(k == 0), stop=False)
        for s in range(4):
            nc.tensor.matmul(R48p[:], UW[s][:, :], Xw[s][:, :],
                             start=(s == 0), stop=False)
        for k in range(2, CH):
            nc.tensor.matmul(L0a[:], U1r[:, k:k + 1], Xn[k][:, 0:512],
                             start=False, stop=(k == CH - 1))
        for k in range(2, CH):
            nc.tensor.matmul(L0b[:], U1r[:, k:k + 1], Xn[k][:, 512:1024],
                             start=False, stop=(k == CH - 1))
        lhs0row = sg.tile([1, T * F], bf16, tag="lhs0row")
        nc.vector.tensor_copy(lhs0row[:, 0:512], L0a[:])
        nc.vector.tensor_copy(lhs0row[:, 512:1024], L0b[:])
        # reshape to (64,16) via 16 rank-1 matmuls against identity rows
        l0Fp = sml.tile([F, T], f32, tag="sml", name="l0fp")
        for t in range(T):
            nc.tensor.matmul(l0Fp[:], lhs0row[0:1, 64 * t:64 * t + 64],
                             I16r[0:1, 16 * t:16 * t + 16],
                             start=(t == 0), stop=(t == T - 1))
        # 0.5 sigmoid-prefactor folded here: scales lhs2T and hence P0
        lhs0F = sg.tile([F, T], bf16, tag="lhs0f")
        nc.vector.tensor_scalar(lhs0F[:], l0Fp[:], 0.5, None, op0=AL.mult)

        # ---- lhs2T chunks (n,16) = U2[:,chunk].T @ lhs0F, packed in one psum
        l2p = sml.tile([P, CH * T], f32, tag="sml", name="l2t")
        for k in range(CH):
            nc.tensor.matmul(l2p[:, k * T:(k + 1) * T],
                             U2[:, k * P:(k + 1) * P], lhs0F[:],
                             start=True, stop=True)
        l2s = sg.tile([P, CH * T], bf16, tag="l2ts")
        nc.vector.tensor_copy(l2s[:], l2p[:])
        lhs2T = [l2s[:, k * T:(k + 1) * T] for k in range(CH)]

        # ---- R48 second half: rows 0:16 rhs3T (Ws3), rows 32:48 rhs_tT (U3)
        for s in range(4, 8):
            nc.tensor.matmul(R48p[:], UW[s][:, :], Xw[s][:, :],
                             start=False, stop=(s == 7))
        R48 = sg.tile([48, N], bf16, tag="r48s")
        nc.scalar.copy(R48[:], R48p[:])

        # ---- rhs_tn chunks: transpose R48[32:48], packed in one psum
        rtp = hlf.tile([P, CH * T], bf16, tag="hlf", name="rtn")
        for k in range(CH):
            nc.tensor.transpose(rtp[:, k * T:(k + 1) * T],
                                R48[32:48, k * P:(k + 1) * P],
                                I128b[32:48, 32:48])
        rts = sg.tile([P, CH * T], bf16, tag="rtns")
        nc.vector.tensor_copy(rts[:], rtp[:])
        rhs_tn = [rts[:, k * T:(k + 1) * T] for k in range(CH)]

        # ---- P0 (16,16) = 0.5*(lhs_t @ rhs_t) + 0.5*be (bias via I16 matmul)
        P0p = sml.tile([T, T], f32, tag="sml", name="p0")
        for k in range(CH):
            nc.tensor.matmul(P0p[:], lhs2T[k][:], rhs_tn[k][:],
                             start=(k == 0), stop=False)
        nc.tensor.matmul(P0p[:], I128b[0:16, 0:16], bePb[:],
                         start=False, stop=True)
        # sig holds tanh(0.5*(P0+be)); sigmoid affine folded into E1T matmul
        # (VeT is 0.5-scaled host-side, hcVe rank-1 term)
        sig = sg.tile([T, T], bf16, tag="sig")
        nc.scalar.activation(sig[:], P0p[:], AF.Tanh)

        # ---- E1^T = sigmoid^T @ Ve^T ; softmax over free dim
        E1Tp = sml.tile([T, T], f32, tag="sml", name="e1t")
        nc.tensor.matmul(E1Tp[:], sig[:], VeT[:], start=True, stop=False)
        nc.tensor.matmul(E1Tp[:], ones1[0:1, 0:16], hcVe[:],
                         start=False, stop=True)
        # values are O(1e-1): skip the max-subtraction for softmax
        sume = sg.tile([T, 1], f32, tag="sume")
        EtT = sg.tile([T, T], bf16, tag="ett")
        nc.scalar.activation(EtT[:], E1Tp[:], AF.Exp,
                             scale=1.0, accum_out=sume[:, 0:1])
        rse = sg.tile([T, 1], f32, tag="rse")
        nc.vector.reciprocal(rse[:], sume[:])
        nc.vector.tensor_scalar(EtT[:], EtT[:], rse[:, 0:1], None, op0=AL.mult)
        Etp = hlf.tile([T, T], bf16, tag="hlf", name="etp")
        nc.tensor.transpose(Etp[:], EtT[:], I128b[0:16, 0:16])
        Et = sg.tile([T, T], bf16, tag="et")
        nc.vector.tensor_copy(Et[:], Etp[:])

        # ---- w1e row (1,16) = Ws1.T @ EtT ; broadcast straight into the
        # pair layout w1Bpair[p=(v,f), s] = w1e[2s+v] with two base-64 matmuls
        w1p = sml.tile([1, T], f32, tag="sml", name="w1p")
        nc.tensor.matmul(w1p[:], Ws1[:], EtT[:], start=True, stop=True)
        w1row = sg.tile([1, T], bf16, tag="w1row")
        nc.scalar.copy(w1row[:], w1p[:])
        w1B2p = sml.tile([P, 8], f32, tag="sml", name="w1b2p")
        nc.tensor.matmul(w1B2p[0:64, :], ones1[0:1, 0:64],
                         w1row[:, 0:T:2], start=True, stop=True)
        nc.tensor.matmul(w1B2p[64:128, :], ones1[0:1, 0:64],
                         w1row[:, 1:T:2], start=True, stop=True)
        # ---- Ws2wP[p=(v,f), s, t] = Ws2d[p,t] * w1e[2s+v]   (128, 8, 16)
        # (reads the w1e broadcast straight from PSUM, saves a copy hop)
        Ws2w = sg.tile([P, 8, T], bf16, tag="ws2w")
        nc.vector.tensor_tensor(
            Ws2w[:],
            Ws2d[:].unsqueeze(1).broadcast_to((P, 8, T)),
            w1B2p[:].unsqueeze(2).broadcast_to((P, 8, T)),
            op=AL.mult)

        # ---- lhs_sT (16, 512) = sum_t1 (Ws2*w1e[t1]).T @ X^T[t1]
        lsTp = sml.tile([T, N], f32, tag="sml", name="lst")
        for s in range(8):
            nc.tensor.matmul(lsTp[:], Ws2w[:, s, :], Xw[s][:, :],
                             start=(s == 0), stop=(s == 7))
        lsT = sg.tile([T, N], bf16, tag="lsts")
        nc.scalar.copy(lsT[:], lsTp[:])

        # ---- rhs_s (16, 512) = Et-weighted rhs3; 0.5 sigmoid-prefactor folded
        rsp = sml.tile([T, N], f32, tag="sml", name="rsp")
        nc.tensor.matmul(rsp[:], Et[:], R48[0:16, :], start=True, stop=True)
        rss = sg.tile([T, N], bf16, tag="rss")
        nc.scalar.mul(rss[:], rsp[:], 0.5)

        # ---- P chunks; bs bias absorbed via identity matmul; G holds
        # tanh(0.5*P + bsh); the sigmoid affine is folded into M1T
        # (VsT 0.5-scaled host-side + vch rank-1 term)
        for k in range(CH):
            Pp = big.tile([P, N], f32, tag="big", name="pp")
            nc.tensor.matmul(Pp[:], lsT[:, k * P:(k + 1) * P], rss[:],
                             start=True, stop=False)
            nc.tensor.matmul(Pp[:], I128b[:], bsh[k][:],
                             start=False, stop=True)
            nc.scalar.activation(G[k][:], Pp[:], AF.Tanh)

        # ---- M1T chunks (c-part, r) + masked softmax -> A1T, dS
        for c in range(CH):
            Mp = big.tile([P, N], f32, tag="big", name="mp")
            for k in range(CH):
                nc.tensor.matmul(Mp[:], G[k][:, c * P:(c + 1) * P], VsT[k][:],
                                 start=(k == 0), stop=False)
            nc.tensor.matmul(Mp[:], ones1[0:1, 0:128], vch[:],
                             start=False, stop=True)
            sme = sg.tile([P, 1], f32, tag=f"sme{c}", name=f"sme{c}")
            nc.scalar.activation(Ex[c][:], Mp[:], AF.Exp,
                                 scale=1.0, accum_out=sme[:, 0:1])
            rcp = sg.tile([P, 1], f32, tag=f"rcp{c}", name=f"rcp{c}")
            nc.vector.reciprocal(rcp[:], sme[:])
            # A1T = (Ex * rcp) * WT   (= S^T o W^T)
            nc.vector.scalar_tensor_tensor(A1T[c][:], Ex[c][:], rcp[:, 0:1],
                                           WT[c][:], op0=AL.mult, op1=AL.mult)
            # diag: dS = sum_r (Ex*rcp)*I over the diagonal block
            dtmp = sg.tile([P, P], bf16, tag="dtmp")
            nc.vector.scalar_tensor_tensor(dtmp[:], Ex[c][:, c * P:(c + 1) * P],
                                           rcp[:, 0:1], I128b[:],
                                           op0=AL.mult, op1=AL.mult)
            nc.vector.tensor_reduce(dSv[c][:], dtmp[:], axis=AX.X, op=AL.add)

        # ---- dS row + broadcast tile (128, 512)
        dSrp = sml.tile([1, N], f32, tag="sml", name="dsrp")
        for c in range(CH):
            nc.tensor.transpose(dSrp[:, c * P:(c + 1) * P], dSv[c][:], I128f[:])
        dSrow = sg.tile([1, N], bf16, tag="dsrow")
        nc.scalar.copy(dSrow[:], dSrp[:])
        dSBp = sml.tile([P, N], f32, tag="sml", name="dsbp")
        nc.tensor.matmul(dSBp[:], ones1[:], dSrow[:], start=True, stop=True)
        nc.scalar.copy(dSB[:], dSBp[:])

        # ---- Tx0 in n-layout (all t at once)
        for k in range(CH):
            nc.vector.tensor_scalar(Tx0n[k][:], Xn[k][:], dSv[k][:, 0:1], None,
                                    op0=AL.mult)

        # =====================================================
        # Cheb + conv + LN, software-pipelined per pair
        # =====================================================
        Tx0P = {}
        TAp = {}
        Tx1T = {}
        ptA = {}
        Tx1n = {}
        TBp = {}
        Tx2T = {}
        TCp = {}
        XhP = {-1: zerot, NP: zerot}
        TDp = {}
        ZT = {}

        def e_tx0p(q):
            t = txp.tile([P, N], bf16, tag="tx0p", name=f"tx0p{q}")
            nc.gpsimd.tensor_tensor(t[:], Xw[q][:], dSB[:], op=AL.mult)
            Tx0P[q] = t

        def e_ta(q):
            p = big.tile([P, N], f32, tag="big", name=f"ta{q}")
            for k in range(CH):
                lhs = Tx0n[k][:, 2 * q * F:(2 * q + 2) * F]
                nc.tensor.matmul(p[:], lhs, A1T[k][:],
                                 start=(k == 0), stop=(k == CH - 1))
            TAp[q] = p

        def e_b(q):
            t = txp.tile([P, N], bf16, tag="tx1t", name=f"tx1t{q}")
            nc.vector.tensor_copy(t[:], TAp[q][:])
            Tx1T[q] = t

        def e_c(q):
            p = hlf.tile([P, N], bf16, tag="hlf", name=f"pta{q}")
            for k in range(CH):
                nc.tensor.transpose(p[:, k * P:(k + 1) * P],
                                    Tx1T[q][:, k * P:(k + 1) * P], I128b[:])
            ptA[q] = p

        def e_d(q):
            t = txp.tile([P, N], bf16, tag="tx1n", name=f"tx1n{q}")
            nc.scalar.copy(t[:], ptA[q][:])
            Tx1n[q] = t

        def e_e(q):
            p = big.tile([P, N], f32, tag="big", name=f"tb{q}")
            for k in range(CH):
                nc.tensor.matmul(p[:], Tx1n[q][:, k * P:(k + 1) * P], WT[k][:],
                                 start=(k == 0), stop=(k == CH - 1))
            TBp[q] = p

        def e_f(q):
            t = txp.tile([P, N], bf16, tag="tx2t", name=f"tx2t{q}")
            nc.vector.scalar_tensor_tensor(t[:], TBp[q][:], 2.0, Tx0P[q][:],
                                           op0=AL.mult, op1=AL.subtract)
            Tx2T[q] = t

        def e_g(q):
            p = big.tile([P, N], f32, tag="big", name=f"tc{q}")
            nc.tensor.matmul(p[:], WcP[0][:], Tx0P[q][:], start=True, stop=False)
            nc.tensor.matmul(p[:], WcP[1][:], Tx1T[q][:], start=False, stop=False)
            nc.tensor.matmul(p[:], WcP[2][:], Tx2T[q][:], start=False, stop=True)
            TCp[q] = p

        def e_h(q):
            t = xhp.tile([P, N], bf16, tag="xh", name=f"xh{q}")
            nc.scalar.activation(t[:], TCp[q][:], AF.Relu, bias=bch[:, 0:1],
                                 scale=1.0)
            XhP[q] = t

        def e_i(q):
            p = big.tile([P, N], f32, tag="big", name=f"td{q}")
            nc.tensor.matmul(p[:], Lprev[:], XhP[q - 1][:], start=True, stop=False)
            nc.tensor.matmul(p[:], Lmid[:], XhP[q][:], start=False, stop=False)
            nc.tensor.matmul(p[:], Lnext[:], XhP[q + 1][:], start=False, stop=False)
            nc.tensor.matmul(p[:], WrP[:], Xw[q][:], start=False, stop=True)
            TDp[q] = p

        def e_j(q):
            t = lnp.tile([P, N], bf16, tag="zt", name=f"zt{q}")
            nc.scalar.activation(t[:], TDp[q][:], AF.Relu, bias=btr[:, 0:1],
                                 scale=1.0)
            ZT[q] = t

        RSTD = {}
        NMR = {}
        SQ = {}
        BASE32 = {}

        def e_sq(q):
            t = lnp.tile([P, N], bf16, tag="sq", name=f"sq{q}")
            nc.vector.tensor_tensor(t[:], ZT[q][:], ZT[q][:], op=AL.mult)
            SQ[q] = t

        def e_ln1(qa, qb):
            # batched stats for a PAIR GROUP: pair qa in rows 0:2, pair qb in
            # rows 32:34 (matmul out base partition must be 0/32/64). One
            # small-op chain then serves two timestep pairs; rows 2:32 are
            # never-read garbage.
            rows = 34 if qb is not None else 2
            sA = sml.tile([34, N], f32, tag="sml", name=f"sA{qa}")
            sB = sml.tile([34, N], f32, tag="sml", name=f"sB{qa}")
            nc.tensor.matmul(sA[0:2, :], B2[:], ZT[qa][:], start=True, stop=True)
            nc.tensor.matmul(sB[0:2, :], B2[:], SQ[qa][:], start=True, stop=True)
            if qb is not None:
                nc.tensor.matmul(sA[32:34, :], B2[:], ZT[qb][:],
                                 start=True, stop=True)
                nc.tensor.matmul(sB[32:34, :], B2[:], SQ[qb][:],
                                 start=True, stop=True)
            mu = lnp.tile([34, N], f32, tag="mu", name=f"mu{qa}")
            nc.scalar.copy(mu[0:rows, :], sA[0:rows, :])
            mu2 = lnp.tile([34, N], f32, tag="mu2", name=f"mu2{qa}")
            # Pool while the pipe is full (throughput), DVE in the drain
            # (latency: Pool elementwise runs at 0.42 efficiency)
            eng = nc.gpsimd if qa == 0 else nc.vector
            eng.tensor_tensor(mu2[0:rows, :], mu[0:rows, :], mu[0:rows, :],
                              op=AL.mult)
            # var+eps = (msq + eps) - mu^2 in one stt
            var = lnp.tile([34, N], f32, tag="var", name=f"var{qa}")
            nc.vector.scalar_tensor_tensor(var[0:rows, :], sB[0:rows, :],
                                           LN_EPS, mu2[0:rows, :],
                                           op0=AL.add, op1=AL.subtract)
            # rstd = sqrt(1/(var+eps)); approx recip is ~18 bits, plenty
            rv = lnp.tile([34, N], f32, tag="rv", name=f"rv{qa}")
            nc.vector.reciprocal_approx_fast(rv[0:rows, :], var[0:rows, :])
            rstd = lnp.tile([34, N], bf16, tag="rstd", name=f"rstd{qa}")
            nc.scalar.activation(rstd[0:rows, :], rv[0:rows, :], AF.Sqrt)
            # nmr = mu * rstd
            nmr = lnp.tile([34, N], bf16, tag="nmr", name=f"nmr{qa}")
            eng.tensor_tensor(nmr[0:rows, :], mu[0:rows, :], rstd[0:rows, :],
                              op=AL.mult)
            RSTD[qa] = rstd[0:2, :]
            NMR[qa] = nmr[0:2, :]
            BASE32[qa] = False
            if qb is not None:
                RSTD[qb] = rstd[32:34, :]
                NMR[qb] = nmr[32:34, :]
                BASE32[qb] = True

        def e_ln2(q):
            z = ZT[q]
            # broadcast to 128 partitions; gamma[f] is folded into B2T
            bt = B2T32 if BASE32[q] else B2T
            rBp = big.tile([P, N], f32, tag="big", name=f"rbp{q}")
            nc.tensor.matmul(rBp[:], bt[:], RSTD[q][:], start=True, stop=True)
            nBp = big.tile([P, N], f32, tag="big", name=f"nbp{q}")
            nc.tensor.matmul(nBp[:], bt[:], NMR[q][:], start=True, stop=True)
            # w = z*(rstd*gam)B + bet - (mu*rstd*gam)B
            u = lnp.tile([P, N], bf16, tag="u", name=f"u{q}")
            nc.vector.tensor_tensor(u[:], z[:], rBp[:], op=AL.mult)
            w = lnp.tile([P, N], bf16, tag="w", name=f"w{q}")
            nc.vector.scalar_tensor_tensor(w[:], u[:], betP[:, 0:1], nBp[:],
                                           op0=AL.add, op1=AL.subtract)
            nc.sync.dma_start(out=ZoutD[q * P:(q + 1) * P, :], in_=w[:])

        # pipeline drive, depth 7: every PE group's inputs are produced in a
        # PREVIOUS iteration, so the in-order PE queue never head-of-line
        # blocks. Stage distances: TA@0, trans@1, TB@2, TC@3, TD@4, ln1@5,
        # ln2@7. Cross-engine hops (stt, relu, copies) happen within the
        # iteration that produced their psum input.
        def live(q):
            return 0 <= q < NP

        for i in range(NP + 7):
            if live(i - 7):
                e_ln2(i - 7)
            if live(i - 1):
                e_c(i - 1)
            if live(i):
                if i == 0:
                    e_tx0p(0)
                e_ta(i)
            if live(i - 1):
                e_d(i - 1)
            if live(i - 2):
                e_e(i - 2)
                e_f(i - 2)
            if live(i - 3):
                e_g(i - 3)
                e_h(i - 3)
            if live(i):
                e_b(i)
                if live(i + 1):
                    e_tx0p(i + 1)
            if live(i - 4):
                e_i(i - 4)
                e_j(i - 4)
                e_sq(i - 4)
            # stats chains: 0..2 individually, (3,4) batched in the drain
            if i in (5, 6, 7):
                e_ln1(i - 5, None)
            elif i == 9:
                e_ln1(3, 4)

    nc.compile()
    return nc


def _host_prep(inputs):
    import ml_dtypes
    bf = ml_dtypes.bfloat16

    X = np.asarray(inputs['X'], np.float32)
    edge_index = np.asarray(inputs['edge_index'])
    U1 = np.asarray(inputs['U1'], np.float32)
    U2 = np.asarray(inputs['U2'], np.float32)
    U3 = np.asarray(inputs['U3'], np.float32)
    be = np.asarray(inputs['be'], np.float32)
    Ve = np.asarray(inputs['Ve'], np.float32)
    Ws1 = np.asarray(inputs['Ws1'], np.float32)
    Ws2 = np.asarray(inputs['Ws2'], np.float32)
    Ws3 = np.asarray(inputs['Ws3'], np.float32)
    bs = np.asarray(inputs['bs'], np.float32)
    Vs = np.asarray(inputs['Vs'], np.float32)
    W_cheb = np.asarray(inputs['W_cheb'], np.float32)
    b_cheb = np.asarray(inputs['b_cheb'], np.float32)
    Wt = np.asarray(inputs['Wt'], np.float32)
    bt = np.asarray(inputs['bt'], np.float32)
    Wr = np.asarray(inputs['Wr'], np.float32)
    br = np.asarray(inputs['br'], np.float32)
    gamma = np.asarray(inputs['gamma'], np.float32)
    beta = np.asarray(inputs['beta'], np.float32)

    # dense symmetric-norm matrix (self-loop +I/-I terms cancel)
    row, col = edge_index[0].astype(np.int64), edge_index[1].astype(np.int64)
    deg = np.zeros(N, np.float32)
    np.add.at(deg, row, 1.0)
    dis = np.where(deg > 0, 1.0 / np.sqrt(np.maximum(deg, 1.0)), 0.0).astype(np.float32)
    wn = -dis[row] * dis[col]
    W = np.zeros((N, N), np.float32)
    np.add.at(W, (row, col), wn)

    # conv block matrices: L[(v,fi),(u,fo)] = Wt[fo,fi,0,dt]
    WtT = [np.ascontiguousarray(Wt[:, :, 0, d].T) for d in range(3)]  # (fi,fo)
    Z64 = np.zeros((F, F), np.float32)
    Lmid = np.block([[WtT[1], WtT[0]], [WtT[2], WtT[1]]]).astype(bf)
    Lprev = np.block([[Z64, Z64], [WtT[0], Z64]]).astype(bf)
    Lnext = np.block([[Z64, WtT[2]], [Z64, Z64]]).astype(bf)
    WrT = np.ascontiguousarray(Wr[:, :, 0, 0].T)
    WrP = np.block([[WrT, Z64], [Z64, WrT]]).astype(bf)
    WcP = np.stack([np.block([[W_cheb[k], Z64], [Z64, W_cheb[k]]]) for k in range(3)]
                   ).astype(bf)

    Wpk = np.stack([WcP[0], WcP[1], WcP[2], Lprev, Lmid, Lnext, WrP])

    Pf = np.zeros((P, PFW), np.float32)
    Pf[:, 0] = np.tile(gamma, 2)
    Pf[:, 1] = np.tile(beta, 2)
    Pf[:, 2] = np.tile(b_cheb, 2)
    Pf[:, 3] = np.tile(bt + br, 2)
    Pf[:, 4:132] = np.eye(P, dtype=np.float32)

    VsTh = 0.5 * np.ascontiguousarray(Vs.T)
    vch = VsTh.sum(axis=0)                 # 0.5*colsum(Vs^T) sigmoid-fold row
    shared = {
        'bsh': (0.5 * bs[0]).astype(bf),
        'VsT': VsTh.astype(bf),
        'WT': np.ascontiguousarray(W.T).astype(bf),
        'Wpk': Wpk,
    }

    in_maps = []
    for core in range(8):
        b, h = core // 2, core % 2
        tmap = list(range(16)) if h == 0 else list(range(6, 16)) + list(range(6))
        Xp = X[b][:, :, tmap]                              # (N, F, 16)
        Xn = np.ascontiguousarray(Xp.transpose(0, 2, 1).reshape(N, T * F)).astype(bf)
        Xw = np.ascontiguousarray(Xp.transpose(2, 1, 0).reshape(8, P, N)).astype(bf)
        UW = np.zeros((8, P, 48), np.float32)
        for tp in range(16):
            s, v = tp // 2, tp % 2
            UW[s, 64 * v:64 * v + 64, tp] = Ws3
            UW[s, 64 * v:64 * v + 64, 32 + tp] = U3
        Pb = np.zeros((P, PBW), np.float32)
        Pb[:, 0:4] = U1.reshape(4, P).T
        Pb[:, 4:20] = np.vstack([Ws2, Ws2])
        VeTh = 0.5 * Ve[np.ix_(tmap, tmap)].T
        Pb[0:16, 20:36] = VeTh
        Pb[0:16, 36] = Ws1[tmap]
        Pb[0, 37:165] = 1.0
        Pb[:, 165:293] = np.eye(P, dtype=np.float32)
        Pb[0:64, 293:805] = U2
        Pb[0, 805:1061] = np.eye(T, dtype=np.float32).reshape(-1)
        # B2: (128, 2) block indicator * 1/64 for per-v mean over f
        Pb[0:64, 1061] = 1.0 / 64
        Pb[64:128, 1062] = 1.0 / 64
        # B2T: (2, 128) block indicator for broadcast back, gamma folded in
        # (replicated at partition 32 for the grouped-stats row-32 slices)
        Pb[0, 1063:1127] = gamma
        Pb[1, 1127:1191] = gamma
        Pb[32, 1063:1127] = gamma
        Pb[33, 1127:1191] = gamma
        # sigmoid-fold rank-1 rows
        Pb[0, 1191:1207] = VeTh.sum(axis=0)
        Pb[0, 1207:1719] = vch
        Pb[0:16, 1719:1735] = 0.5 * be[0][np.ix_(tmap, tmap)]
        Pfc = Pf.copy()
        Pfc[0:16, 132:148] = 0.5 * be[0][np.ix_(tmap, tmap)]
        m = dict(shared)
        m.update({
            'Xn': Xn, 'Xw': Xw, 'UW': UW.astype(bf),
            'Pb': Pb.astype(bf), 'Pf': Pfc,
        })
        in_maps.append(m)
    return in_maps


def kernel(**inputs):
    import sys
    if '/opt/trn_rl_repo' not in sys.path:
        sys.path.insert(0, '/opt/trn_rl_repo')
    from concourse.bass_utils import run_bass_kernel_spmd

    if 'nc' not in _CACHE:
        _CACHE['nc'] = _build_program()
    nc = _CACHE['nc']

    in_maps = _host_prep(inputs)
    res = run_bass_kernel_spmd(nc, in_maps, list(range(8)))
    out = np.zeros((B, N, F, T), np.float32)
    for core in range(8):
        b, h = core // 2, core % 2
        Z = np.asarray(res.results[core]['Zout']).astype(np.float32)
        # rows q*128 + v*64 + f, cols n  ->  (n, f, slot=2q+v)
        Zs = Z.reshape(NP, 2, F, N).transpose(3, 2, 0, 1).reshape(N, F, NSLOT)
        wstart = 0 if h == 0 else 6
        jlo = 0 if h == 0 else 2
        out[b, :, :, wstart + jlo:wstart + jlo + 8] = Zs[:, :, jlo:jlo + 8]
    return out


# revision 66
# speedup vs baseline: 1.0422x; 1.0102x over previous
"""ASTGCN block Trainium2 kernel (v2).

Strategy: 8 cores; core c handles batch b = c//2, time-half h = c%2 (8 output
timesteps each, data-parallel over B and T). Attention (temporal Et, spatial
S) is per-b and replicated on the 2 cores sharing a b. The sparse graph
propagation is reformulated as dense (N,N) matmuls: the edge-scatter of the
symmetric norm is accumulated host-side into a dense W (the +I/-I self-loop
terms cancel), so  prop1(h) = (W*S) @ h  and  prop2(h) = W @ h.

v2 changes vs baseline:
- Input DMAs ordered by first use (Pb/Pf/Xn first) and X tensors split in
  halves so attention matmuls start ~5us in instead of after all loads.
- Single activation-table regime: sigmoid via tanh (0.5*tanh(x/2)+0.5, in
  the exp table) and LN rstd via exp(-0.5*ln(var+eps)); only one table
  switch in the whole program (exp_and_others -> natural_log_exp...).
- LayerNorm runs in pair layout: per-pair stats via ones-block matmuls on
  PE (reduce over the f partition rows), rstd/-mu*rstd broadcast back with
  block matmuls; no transposes of the conv output at all.
- Output stored in pair layout as bf16; host does the final (f,n) -> (n,f)
  transpose and fp32 upcast.
- cheb -> conv -> LN -> store software-pipelined across the 5 timestep
  pairs to keep PE dense (p-state) and overlap store DMAs with compute.

Per-core time axis is PERMUTED so the program is identical SPMD: slot t' maps
to global t via tmap (identity for h=0, rotated by 6 for h=1); all
t-dependent weights (be, Ve, Ws1, UW) are permuted host-side to match.
"""

import numpy as np

B, N, F, T = 4, 512, 64, 16
P = 128
CH = N // P            # 4 n-chunks
NSLOT = 10             # cheb window timesteps per core (5 pairs)
NP = NSLOT // 2        # 5 pairs
LN_EPS = 1e-5

PBW = 1735             # packed bf16 constant width
PFW = 148              # packed f32 constant width

_CACHE = {}


def _build_program():
    import sys
    if '/opt/trn_rl_repo' not in sys.path:
        sys.path.insert(0, '/opt/trn_rl_repo')
    from contextlib import ExitStack
    import concourse.bass as bass
    import concourse.tile as tile
    from concourse import bacc, mybir

    dt = mybir.dt
    AL = mybir.AluOpType
    AF = mybir.ActivationFunctionType
    AX = mybir.AxisListType
    f32 = dt.float32
    bf16 = dt.bfloat16

    nc = bacc.Bacc("TRN2", target_bir_lowering=False, debug=False, num_devices=1)

    def din(name, shape, d=bf16):
        return nc.dram_tensor(name, list(shape), d, kind="ExternalInput").ap()

    XnD   = din("Xn", (N, T * F))
    XwD   = din("Xw", (8, P, N))
    UWD   = din("UW", (8, P, 48))
    bshD  = din("bsh", (N, N))          # 0.5 * bs
    VsTD  = din("VsT", (N, N))
    WTD   = din("WT", (N, N))
    WpkD  = din("Wpk", (7, P, P))
    PbD   = din("Pb", (P, PBW))
    PfD   = din("Pf", (P, PFW), f32)
    ZoutD = nc.dram_tensor("Zout", [NP * P, N], bf16, kind="ExternalOutput").ap()

    with tile.TileContext(nc) as tc, ExitStack() as ctx:
        sg = ctx.enter_context(tc.tile_pool(name="sg", bufs=1))
        big = ctx.enter_context(tc.tile_pool(name="big", bufs=5, space="PSUM"))
        sml = ctx.enter_context(tc.tile_pool(name="sml", bufs=2, space="PSUM"))
        hlf = ctx.enter_context(tc.tile_pool(name="hlf", bufs=1, space="PSUM"))
        xhp = ctx.enter_context(tc.tile_pool(name="xhp", bufs=7))
        txp = ctx.enter_context(tc.tile_pool(name="txp", bufs=5))
        lnp = ctx.enter_context(tc.tile_pool(name="lnp", bufs=5))

        # ------------- input DMAs, ordered by first use -------------
        Pb = sg.tile([P, PBW], bf16, tag="pb")
        nc.sync.dma_start(out=Pb[:], in_=PbD)
        Pf = sg.tile([P, PFW], f32, tag="pf")
        nc.sync.dma_start(out=Pf[:], in_=PfD)
        XnA = sg.tile([P, 2, T * F], bf16, tag="xna")
        XnB = sg.tile([P, 2, T * F], bf16, tag="xnb")
        XnDr = XnD.rearrange("(k p) t -> p k t", k=CH)
        UWAll = sg.tile([P, 8, 48], bf16, tag="uwall")
        XwA = sg.tile([P, 4, N], bf16, tag="xwa")
        XwB = sg.tile([P, 4, N], bf16, tag="xwb")
        XwDr = XwD.rearrange("s p n -> p s n")
        # interleave the two X layouts so both attention input paths
        # (lhs0 over Xn, R48 over Xw) can start on half the data
        nc.sync.dma_start(out=XnA[:], in_=XnDr[:, 0:2, :])
        nc.sync.dma_start(out=UWAll[:], in_=UWD.rearrange("s p n -> p s n"))
        nc.sync.dma_start(out=XwA[:], in_=XwDr[:, 0:4, :])
        nc.sync.dma_start(out=XnB[:], in_=XnDr[:, 2:4, :])
        nc.sync.dma_start(out=XwB[:], in_=XwDr[:, 4:8, :])
        bsAll = sg.tile([P, CH, N], bf16, tag="bsall")
        nc.sync.dma_start(out=bsAll[:], in_=bshD.rearrange("(k p) n -> p k n", k=CH))
        VsTAll = sg.tile([P, CH, N], bf16, tag="vstall")
        nc.sync.dma_start(out=VsTAll[:], in_=VsTD.rearrange("(k p) n -> p k n", k=CH))
        WTAll = sg.tile([P, CH, N], bf16, tag="wtall")
        nc.sync.dma_start(out=WTAll[:], in_=WTD.rearrange("(k p) n -> p k n", k=CH))
        Wpk = sg.tile([P, 7, P], bf16, tag="wpk")
        nc.sync.dma_start(out=Wpk[:], in_=WpkD.rearrange("w p c -> p w c"))

        Xn = [XnA[:, 0, :], XnA[:, 1, :], XnB[:, 0, :], XnB[:, 1, :]]
        Xw = [XwA[:, s, :] for s in range(4)] + [XwB[:, s, :] for s in range(4)]
        UW = [UWAll[:, s, :] for s in range(8)]
        bsh = [bsAll[:, k, :] for k in range(CH)]
        VsT = [VsTAll[:, k, :] for k in range(CH)]
        WT = [WTAll[:, k, :] for k in range(CH)]
        WcP = [Wpk[:, k, :] for k in range(3)]
        Lprev, Lmid, Lnext, WrP = (Wpk[:, 3, :], Wpk[:, 4, :], Wpk[:, 5, :],
                                   Wpk[:, 6, :])
        # packed bf16 layout
        U1r = Pb[:, 0:4]
        Ws2d = Pb[:, 4:20]
        VeT = Pb[0:16, 20:36]
        Ws1 = Pb[0:16, 36:37]
        ones1 = Pb[0:1, 37:165]
        I128b = Pb[:, 165:293]
        U2 = Pb[0:64, 293:805]
        I16r = Pb[0:1, 805:1061]     # I16 rows flattened: e_t = [0:1, 16t:16t+16]
        B2 = Pb[:, 1061:1063]        # (128,2) block col-indicator * 1/64
        B2T = Pb[0:2, 1063:1191]     # (2,128) block row-indicator * gamma[f]
        B2T32 = Pb[32:34, 1063:1191]  # same rows replicated at partition 32
        hcVe = Pb[0:1, 1191:1207]    # 0.5*colsum(VeT')  [sigmoid-fold row]
        vch = Pb[0:1, 1207:1719]     # 0.5*colsum(VsT')  [sigmoid-fold row]
        bePb = Pb[0:16, 1719:1735]   # 0.5*be (permuted), bf16
        # packed f32 layout
        gamP = Pf[:, 0:1]
        betP = Pf[:, 1:2]
        bch = Pf[:, 2:3]
        btr = Pf[:, 3:4]
        I128f = Pf[:, 4:132]
        bePh = Pf[0:16, 132:148]     # 0.5 * be (permuted)

        zerot = sg.tile([P, N], bf16, tag="zerot")
        nc.vector.memset(zerot[:], 0.0)
        epsP = sg.tile([P, 1], f32, tag="epsP")
        nc.vector.memset(epsP[:], LN_EPS)

        # persistent sbuf intermediates
        G = [sg.tile([P, N], bf16, tag=f"g{k}", name=f"g{k}") for k in range(CH)]
        Ex = [sg.tile([P, N], bf16, tag=f"ex{k}", name=f"ex{k}") for k in range(CH)]
        A1T = [sg.tile([P, N], bf16, tag=f"a1t{k}", name=f"a1t{k}") for k in range(CH)]
        dSv = [sg.tile([P, 1], f32, tag=f"dsv{k}", name=f"dsv{k}") for k in range(CH)]
        Tx0n = [sg.tile([P, T * F], bf16, tag=f"tx0n{k}", name=f"tx0n{k}")
                for k in range(CH)]
        dSB = sg.tile([P, N], bf16, tag="dsb")

        # =====================================================
        # Attention phase
        # =====================================================
        # ---- lhs0[(t,f)] = sum_n U1[n] X[n,(t,f)]  -> (1,1024)
        # accumulation interleaved with the R48 first half so PE follows the
        # XnA / XwA / XnB / XwB DMA arrival order
        L0a = sml.tile([1, 512], f32, tag="sml", name="l0a")
        L0b = sml.tile([1, 512], f32, tag="sml", name="l0b")
        R48p = hlf.tile([48, N], f32, tag="hlf", name="r48")
        for k in range(2):
            nc.tensor.matmul(L0a[:], U1r[:, k:k + 1], Xn[k][:, 0:512],
                             start=(k == 0), stop=False)
        for k in range(2):
            nc.tensor.matmul(L0b[:], U1r[:, k:k + 1], Xn[k][:, 512:1024],
                             start=(k == 0), stop=False)
        for s in range(4):
            nc.tensor.matmul(R48p[:], UW[s][:, :], Xw[s][:, :],
                             start=(s == 0), stop=False)
        for k in range(2, CH):
            nc.tensor.matmul(L0a[:], U1r[:, k:k + 1], Xn[k][:, 0:512],
                             start=False, stop=(k == CH - 1))
        for k in range(2, CH):
            nc.tensor.matmul(L0b[:], U1r[:, k:k + 1], Xn[k][:, 512:1024],
                             start=False, stop=(k == CH - 1))
        lhs0row = sg.tile([1, T * F], bf16, tag="lhs0row")
        nc.vector.tensor_copy(lhs0row[:, 0:512], L0a[:])
        nc.vector.tensor_copy(lhs0row[:, 512:1024], L0b[:])
        # reshape to (64,16) via 16 rank-1 matmuls against identity rows
        l0Fp = sml.tile([F, T], f32, tag="sml", name="l0fp")
        for t in range(T):
            nc.tensor.matmul(l0Fp[:], lhs0row[0:1, 64 * t:64 * t + 64],
                             I16r[0:1, 16 * t:16 * t + 16],
                             start=(t == 0), stop=(t == T - 1))
        # 0.5 sigmoid-prefactor folded here: scales lhs2T and hence P0
        lhs0F = sg.tile([F, T], bf16, tag="lhs0f")
        nc.vector.tensor_scalar(lhs0F[:], l0Fp[:], 0.5, None, op0=AL.mult)

        # ---- lhs2T chunks (n,16) = U2[:,chunk].T @ lhs0F, packed in one psum
        l2p = sml.tile([P, CH * T], f32, tag="sml", name="l2t")
        for k in range(CH):
            nc.tensor.matmul(l2p[:, k * T:(k + 1) * T],
                             U2[:, k * P:(k + 1) * P], lhs0F[:],
                             start=True, stop=True)
        l2s = sg.tile([P, CH * T], bf16, tag="l2ts")
        nc.vector.tensor_copy(l2s[:], l2p[:])
        lhs2T = [l2s[:, k * T:(k + 1) * T] for k in range(CH)]

        # ---- R48 second half: rows 0:16 rhs3T (Ws3), rows 32:48 rhs_tT (U3)
        for s in range(4, 8):
            nc.tensor.matmul(R48p[:], UW[s][:, :], Xw[s][:, :],
                             start=False, stop=(s == 7))
        R48 = sg.tile([48, N], bf16, tag="r48s")
        nc.scalar.copy(R48[:], R48p[:])

        # ---- rhs_tn chunks: transpose R48[32:48], packed in one psum
        rtp = hlf.tile([P, CH * T], bf16, tag="hlf", name="rtn")
        for k in range(CH):
            nc.tensor.transpose(rtp[:, k * T:(k + 1) * T],
                                R48[32:48, k * P:(k + 1) * P],
                                I128b[32:48, 32:48])
        rts = sg.tile([P, CH * T], bf16, tag="rtns")
        nc.vector.tensor_copy(rts[:], rtp[:])
        rhs_tn = [rts[:, k * T:(k + 1) * T] for k in range(CH)]

        # ---- P0 (16,16) = 0.5*(lhs_t @ rhs_t) + 0.5*be (bias via I16 matmul)
        P0p = sml.tile([T, T], f32, tag="sml", name="p0")
        for k in range(CH):
            nc.tensor.matmul(P0p[:], lhs2T[k][:], rhs_tn[k][:],
                             start=(k == 0), stop=False)
        nc.tensor.matmul(P0p[:], I128b[0:16, 0:16], bePb[:],
                         start=False, stop=True)
        # sig holds tanh(0.5*(P0+be)); sigmoid affine folded into E1T matmul
        # (VeT is 0.5-scaled host-side, hcVe rank-1 term)
        sig = sg.tile([T, T], bf16, tag="sig")
        nc.scalar.activation(sig[:], P0p[:], AF.Tanh)

        # ---- E1^T = sigmoid^T @ Ve^T ; softmax over free dim
        E1Tp = sml.tile([T, T], f32, tag="sml", name="e1t")
        nc.tensor.matmul(E1Tp[:], sig[:], VeT[:], start=True, stop=False)
        nc.tensor.matmul(E1Tp[:], ones1[0:1, 0:16], hcVe[:],
                         start=False, stop=True)
        # values are O(1e-1): skip the max-subtraction for softmax
        sume = sg.tile([T, 1], f32, tag="sume")
        EtT = sg.tile([T, T], bf16, tag="ett")
        nc.scalar.activation(EtT[:], E1Tp[:], AF.Exp,
                             scale=1.0, accum_out=sume[:, 0:1])
        rse = sg.tile([T, 1], f32, tag="rse")
        nc.vector.reciprocal(rse[:], sume[:])
        nc.vector.tensor_scalar(EtT[:], EtT[:], rse[:, 0:1], None, op0=AL.mult)
        Etp = hlf.tile([T, T], bf16, tag="hlf", name="etp")
        nc.tensor.transpose(Etp[:], EtT[:], I128b[0:16, 0:16])
        Et = sg.tile([T, T], bf16, tag="et")
        nc.vector.tensor_copy(Et[:], Etp[:])

        # ---- w1e row (1,16) = Ws1.T @ EtT ; broadcast straight into the
        # pair layout w1Bpair[p=(v,f), s] = w1e[2s+v] with two base-64 matmuls
        w1p = sml.tile([1, T], f32, tag="sml", name="w1p")
        nc.tensor.matmul(w1p[:], Ws1[:], EtT[:], start=True, stop=True)
        w1row = sg.tile([1, T], bf16, tag="w1row")
        nc.scalar.copy(w1row[:], w1p[:])
        w1B2p = sml.tile([P, 8], f32, tag="sml", name="w1b2p")
        nc.tensor.matmul(w1B2p[0:64, :], ones1[0:1, 0:64],
                         w1row[:, 0:T:2], start=True, stop=True)
        nc.tensor.matmul(w1B2p[64:128, :], ones1[0:1, 0:64],
                         w1row[:, 1:T:2], start=True, stop=True)
        # ---- Ws2wP[p=(v,f), s, t] = Ws2d[p,t] * w1e[2s+v]   (128, 8, 16)
        # (reads the w1e broadcast straight from PSUM, saves a copy hop)
        Ws2w = sg.tile([P, 8, T], bf16, tag="ws2w")
        nc.vector.tensor_tensor(
            Ws2w[:],
            Ws2d[:].unsqueeze(1).broadcast_to((P, 8, T)),
            w1B2p[:].unsqueeze(2).broadcast_to((P, 8, T)),
            op=AL.mult)

        # ---- lhs_sT (16, 512) = sum_t1 (Ws2*w1e[t1]).T @ X^T[t1]
        lsTp = sml.tile([T, N], f32, tag="sml", name="lst")
        for s in range(8):
            nc.tensor.matmul(lsTp[:], Ws2w[:, s, :], Xw[s][:, :],
                             start=(s == 0), stop=(s == 7))
        lsT = sg.tile([T, N], bf16, tag="lsts")
        nc.scalar.copy(lsT[:], lsTp[:])

        # ---- rhs_s (16, 512) = Et-weighted rhs3; 0.5 sigmoid-prefactor folded
        rsp = sml.tile([T, N], f32, tag="sml", name="rsp")
        nc.tensor.matmul(rsp[:], Et[:], R48[0:16, :], start=True, stop=True)
        rss = sg.tile([T, N], bf16, tag="rss")
        nc.scalar.mul(rss[:], rsp[:], 0.5)

        # ---- P chunks; bs bias absorbed via identity matmul; G holds
        # tanh(0.5*P + bsh); the sigmoid affine is folded into M1T
        # (VsT 0.5-scaled host-side + vch rank-1 term)
        for k in range(CH):
            Pp = big.tile([P, N], f32, tag="big", name="pp")
            nc.tensor.matmul(Pp[:], lsT[:, k * P:(k + 1) * P], rss[:],
                             start=True, stop=False)
            nc.tensor.matmul(Pp[:], I128b[:], bsh[k][:],
                             start=False, stop=True)
            nc.scalar.activation(G[k][:], Pp[:], AF.Tanh)

        # ---- M1T chunks (c-part, r) + masked softmax -> A1T, dS
        for c in range(CH):
            Mp = big.tile([P, N], f32, tag="big", name="mp")
            for k in range(CH):
                nc.tensor.matmul(Mp[:], G[k][:, c * P:(c + 1) * P], VsT[k][:],
                                 start=(k == 0), stop=False)
            nc.tensor.matmul(Mp[:], ones1[0:1, 0:128], vch[:],
                             start=False, stop=True)
            sme = sg.tile([P, 1], f32, tag=f"sme{c}", name=f"sme{c}")
            nc.scalar.activation(Ex[c][:], Mp[:], AF.Exp,
                                 scale=1.0, accum_out=sme[:, 0:1])
            rcp = sg.tile([P, 1], f32, tag=f"rcp{c}", name=f"rcp{c}")
            nc.vector.reciprocal(rcp[:], sme[:])
            # A1T = (Ex * rcp) * WT   (= S^T o W^T)
            nc.vector.scalar_tensor_tensor(A1T[c][:], Ex[c][:], rcp[:, 0:1],
                                           WT[c][:], op0=AL.mult, op1=AL.mult)
            # diag: dS = sum_r (Ex*rcp)*I over the diagonal block
            dtmp = sg.tile([P, P], bf16, tag="dtmp")
            nc.vector.scalar_tensor_tensor(dtmp[:], Ex[c][:, c * P:(c + 1) * P],
                                           rcp[:, 0:1], I128b[:],
                                           op0=AL.mult, op1=AL.mult)
            nc.vector.tensor_reduce(dSv[c][:], dtmp[:], axis=AX.X, op=AL.add)

        # ---- dS row + broadcast tile (128, 512)
        dSrp = sml.tile([1, N], f32, tag="sml", name="dsrp")
        for c in range(CH):
            nc.tensor.transpose(dSrp[:, c * P:(c + 1) * P], dSv[c][:], I128f[:])
        dSrow = sg.tile([1, N], bf16, tag="dsrow")
        nc.scalar.copy(dSrow[:], dSrp[:])
        dSBp = sml.tile([P, N], f32, tag="sml", name="dsbp")
        nc.tensor.matmul(dSBp[:], ones1[:], dSrow[:], start=True, stop=True)
        nc.scalar.copy(dSB[:], dSBp[:])

        # ---- Tx0 in n-layout (all t at once)
        for k in range(CH):
            nc.vector.tensor_scalar(Tx0n[k][:], Xn[k][:], dSv[k][:, 0:1], None,
                                    op0=AL.mult)

        # =====================================================
        # Cheb + conv + LN, software-pipelined per pair
        # =====================================================
        Tx0P = {}
        TAp = {}
        Tx1T = {}
        ptA = {}
        Tx1n = {}
        TBp = {}
        Tx2T = {}
        TCp = {}
        XhP = {-1: zerot, NP: zerot}
        TDp = {}
        ZT = {}

        def e_tx0p(q):
            t = txp.tile([P, N], bf16, tag="tx0p", name=f"tx0p{q}")
            nc.gpsimd.tensor_tensor(t[:], Xw[q][:], dSB[:], op=AL.mult)
            Tx0P[q] = t

        def e_ta(q):
            p = big.tile([P, N], f32, tag="big", name=f"ta{q}")
            for k in range(CH):
                lhs = Tx0n[k][:, 2 * q * F:(2 * q + 2) * F]
                nc.tensor.matmul(p[:], lhs, A1T[k][:],
                                 start=(k == 0), stop=(k == CH - 1))
            TAp[q] = p

        def e_b(q):
            t = txp.tile([P, N], bf16, tag="tx1t", name=f"tx1t{q}")
            nc.vector.tensor_copy(t[:], TAp[q][:])
            Tx1T[q] = t

        def e_c(q):
            p = hlf.tile([P, N], bf16, tag="hlf", name=f"pta{q}")
            for k in range(CH):
                nc.tensor.transpose(p[:, k * P:(k + 1) * P],
                                    Tx1T[q][:, k * P:(k + 1) * P], I128b[:])
            ptA[q] = p

        def e_d(q):
            t = txp.tile([P, N], bf16, tag="tx1n", name=f"tx1n{q}")
            nc.scalar.copy(t[:], ptA[q][:])
            Tx1n[q] = t

        def e_e(q):
            p = big.tile([P, N], f32, tag="big", name=f"tb{q}")
            for k in range(CH):
                nc.tensor.matmul(p[:], Tx1n[q][:, k * P:(k + 1) * P], WT[k][:],
                                 start=(k == 0), stop=(k == CH - 1))
            TBp[q] = p

        def e_f(q):
            t = txp.tile([P, N], bf16, tag="tx2t", name=f"tx2t{q}")
            nc.vector.scalar_tensor_tensor(t[:], TBp[q][:], 2.0, Tx0P[q][:],
                                           op0=AL.mult, op1=AL.subtract)
            Tx2T[q] = t

        def e_g(q):
            p = big.tile([P, N], f32, tag="big", name=f"tc{q}")
            nc.tensor.matmul(p[:], WcP[0][:], Tx0P[q][:], start=True, stop=False)
            nc.tensor.matmul(p[:], WcP[1][:], Tx1T[q][:], start=False, stop=False)
            nc.tensor.matmul(p[:], WcP[2][:], Tx2T[q][:], start=False, stop=True)
            TCp[q] = p

        def e_h(q):
            t = xhp.tile([P, N], bf16, tag="xh", name=f"xh{q}")
            nc.scalar.activation(t[:], TCp[q][:], AF.Relu, bias=bch[:, 0:1],
                                 scale=1.0)
            XhP[q] = t

        def e_i(q):
            p = big.tile([P, N], f32, tag="big", name=f"td{q}")
            nc.tensor.matmul(p[:], Lprev[:], XhP[q - 1][:], start=True, stop=False)
            nc.tensor.matmul(p[:], Lmid[:], XhP[q][:], start=False, stop=False)
            nc.tensor.matmul(p[:], Lnext[:], XhP[q + 1][:], start=False, stop=False)
            nc.tensor.matmul(p[:], WrP[:], Xw[q][:], start=False, stop=True)
            TDp[q] = p

        def e_j(q):
            t = lnp.tile([P, N], bf16, tag="zt", name=f"zt{q}")
            nc.scalar.activation(t[:], TDp[q][:], AF.Relu, bias=btr[:, 0:1],
                                 scale=1.0)
            ZT[q] = t

        RSTD = {}
        NMR = {}
        SQ = {}
        BASE32 = {}

        def e_sq(q):
            t = lnp.tile([P, N], bf16, tag="sq", name=f"sq{q}")
            nc.vector.tensor_tensor(t[:], ZT[q][:], ZT[q][:], op=AL.mult)
            SQ[q] = t

        def e_ln1(qa, qb):
            # batched stats for a PAIR GROUP: pair qa in rows 0:2, pair qb in
            # rows 32:34 (matmul out base partition must be 0/32/64). One
            # small-op chain then serves two timestep pairs; rows 2:32 are
            # never-read garbage.
            rows = 34 if qb is not None else 2
            sA = sml.tile([34, N], f32, tag="sml", name=f"sA{qa}")
            sB = sml.tile([34, N], f32, tag="sml", name=f"sB{qa}")
            nc.tensor.matmul(sA[0:2, :], B2[:], ZT[qa][:], start=True, stop=True)
            nc.tensor.matmul(sB[0:2, :], B2[:], SQ[qa][:], start=True, stop=True)
            if qb is not None:
                nc.tensor.matmul(sA[32:34, :], B2[:], ZT[qb][:],
                                 start=True, stop=True)
                nc.tensor.matmul(sB[32:34, :], B2[:], SQ[qb][:],
                                 start=True, stop=True)
            mu = lnp.tile([34, N], f32, tag="mu", name=f"mu{qa}")
            nc.scalar.copy(mu[0:rows, :], sA[0:rows, :])
            mu2 = lnp.tile([34, N], f32, tag="mu2", name=f"mu2{qa}")
            # Pool while the pipe is full (throughput), DVE in the drain
            # (latency: Pool elementwise runs at 0.42 efficiency)
            eng = nc.gpsimd if qa == 0 else nc.vector
            eng.tensor_tensor(mu2[0:rows, :], mu[0:rows, :], mu[0:rows, :],
                              op=AL.mult)
            # var+eps = (msq + eps) - mu^2 in one stt
            var = lnp.tile([34, N], f32, tag="var", name=f"var{qa}")
            nc.vector.scalar_tensor_tensor(var[0:rows, :], sB[0:rows, :],
                                           LN_EPS, mu2[0:rows, :],
                                           op0=AL.add, op1=AL.subtract)
            # rstd = sqrt(1/(var+eps)); approx recip is ~18 bits, plenty
            rv = lnp.tile([34, N], f32, tag="rv", name=f"rv{qa}")
            nc.vector.reciprocal_approx_fast(rv[0:rows, :], var[0:rows, :])
            rstd = lnp.tile([34, N], bf16, tag="rstd", name=f"rstd{qa}")
            nc.scalar.activation(rstd[0:rows, :], rv[0:rows, :], AF.Sqrt)
            # nmr = mu * rstd
            nmr = lnp.tile([34, N], bf16, tag="nmr", name=f"nmr{qa}")
            eng.tensor_tensor(nmr[0:rows, :], mu[0:rows, :], rstd[0:rows, :],
                              op=AL.mult)
            RSTD[qa] = rstd[0:2, :]
            NMR[qa] = nmr[0:2, :]
            BASE32[qa] = False
            if qb is not None:
                RSTD[qb] = rstd[32:34, :]
                NMR[qb] = nmr[32:34, :]
                BASE32[qb] = True

        def e_ln2(q):
            z = ZT[q]
            # broadcast to 128 partitions; gamma[f] is folded into B2T
            bt = B2T32 if BASE32[q] else B2T
            rBp = big.tile([P, N], f32, tag="big", name=f"rbp{q}")
            nc.tensor.matmul(rBp[:], bt[:], RSTD[q][:], start=True, stop=True)
            nBp = big.tile([P, N], f32, tag="big", name=f"nbp{q}")
            nc.tensor.matmul(nBp[:], bt[:], NMR[q][:], start=True, stop=True)
            # w = z*(rstd*gam)B + bet - (mu*rstd*gam)B
            u = lnp.tile([P, N], bf16, tag="u", name=f"u{q}")
            nc.vector.tensor_tensor(u[:], z[:], rBp[:], op=AL.mult)
            w = lnp.tile([P, N], bf16, tag="w", name=f"w{q}")
            nc.vector.scalar_tensor_tensor(w[:], u[:], betP[:, 0:1], nBp[:],
                                           op0=AL.add, op1=AL.subtract)
            nc.sync.dma_start(out=ZoutD[q * P:(q + 1) * P, :], in_=w[:])

        # pipeline drive, depth 7: every PE group's inputs are produced in a
        # PREVIOUS iteration, so the in-order PE queue never head-of-line
        # blocks. Stage distances: TA@0, trans@1, TB@2, TC@3, TD@4, ln1@5,
        # ln2@7. Cross-engine hops (stt, relu, copies) happen within the
        # iteration that produced their psum input.
        def live(q):
            return 0 <= q < NP

        LN2_AT = {7: [0], 8: [1], 9: [2, 3], 10: [4]}
        for i in range(NP + 6):
            for q2 in LN2_AT.get(i, []):
                e_ln2(q2)
            if live(i - 1):
                e_c(i - 1)
            if live(i):
                if i == 0:
                    e_tx0p(0)
                e_ta(i)
            if live(i - 1):
                e_d(i - 1)
            if live(i - 2):
                e_e(i - 2)
                e_f(i - 2)
            if live(i - 3):
                e_g(i - 3)
                e_h(i - 3)
            if live(i):
                e_b(i)
                if live(i + 1):
                    e_tx0p(i + 1)
            if live(i - 4):
                e_i(i - 4)
                e_j(i - 4)
                e_sq(i - 4)
            # stats chains: 0..2 individually, (3,4) batched in the drain
            if i in (5, 6, 7):
                e_ln1(i - 5, None)
            elif i == 8:
                e_ln1(3, 4)

    nc.compile()
    return nc


def _host_prep(inputs):
    import ml_dtypes
    bf = ml_dtypes.bfloat16

    X = np.asarray(inputs['X'], np.float32)
    edge_index = np.asarray(inputs['edge_index'])
    U1 = np.asarray(inputs['U1'], np.float32)
    U2 = np.asarray(inputs['U2'], np.float32)
    U3 = np.asarray(inputs['U3'], np.float32)
    be = np.asarray(inputs['be'], np.float32)
    Ve = np.asarray(inputs['Ve'], np.float32)
    Ws1 = np.asarray(inputs['Ws1'], np.float32)
    Ws2 = np.asarray(inputs['Ws2'], np.float32)
    Ws3 = np.asarray(inputs['Ws3'], np.float32)
    bs = np.asarray(inputs['bs'], np.float32)
    Vs = np.asarray(inputs['Vs'], np.float32)
    W_cheb = np.asarray(inputs['W_cheb'], np.float32)
    b_cheb = np.asarray(inputs['b_cheb'], np.float32)
    Wt = np.asarray(inputs['Wt'], np.float32)
    bt = np.asarray(inputs['bt'], np.float32)
    Wr = np.asarray(inputs['Wr'], np.float32)
    br = np.asarray(inputs['br'], np.float32)
    gamma = np.asarray(inputs['gamma'], np.float32)
    beta = np.asarray(inputs['beta'], np.float32)

    # dense symmetric-norm matrix (self-loop +I/-I terms cancel)
    row, col = edge_index[0].astype(np.int64), edge_index[1].astype(np.int64)
    deg = np.zeros(N, np.float32)
    np.add.at(deg, row, 1.0)
    dis = np.where(deg > 0, 1.0 / np.sqrt(np.maximum(deg, 1.0)), 0.0).astype(np.float32)
    wn = -dis[row] * dis[col]
    W = np.zeros((N, N), np.float32)
    np.add.at(W, (row, col), wn)

    # conv block matrices: L[(v,fi),(u,fo)] = Wt[fo,fi,0,dt]
    WtT = [np.ascontiguousarray(Wt[:, :, 0, d].T) for d in range(3)]  # (fi,fo)
    Z64 = np.zeros((F, F), np.float32)
    Lmid = np.block([[WtT[1], WtT[0]], [WtT[2], WtT[1]]]).astype(bf)
    Lprev = np.block([[Z64, Z64], [WtT[0], Z64]]).astype(bf)
    Lnext = np.block([[Z64, WtT[2]], [Z64, Z64]]).astype(bf)
    WrT = np.ascontiguousarray(Wr[:, :, 0, 0].T)
    WrP = np.block([[WrT, Z64], [Z64, WrT]]).astype(bf)
    WcP = np.stack([np.block([[W_cheb[k], Z64], [Z64, W_cheb[k]]]) for k in range(3)]
                   ).astype(bf)

    Wpk = np.stack([WcP[0], WcP[1], WcP[2], Lprev, Lmid, Lnext, WrP])

    Pf = np.zeros((P, PFW), np.float32)
    Pf[:, 0] = np.tile(gamma, 2)
    Pf[:, 1] = np.tile(beta, 2)
    Pf[:, 2] = np.tile(b_cheb, 2)
    Pf[:, 3] = np.tile(bt + br, 2)
    Pf[:, 4:132] = np.eye(P, dtype=np.float32)

    VsTh = 0.5 * np.ascontiguousarray(Vs.T)
    vch = VsTh.sum(axis=0)                 # 0.5*colsum(Vs^T) sigmoid-fold row
    shared = {
        'bsh': (0.5 * bs[0]).astype(bf),
        'VsT': VsTh.astype(bf),
        'WT': np.ascontiguousarray(W.T).astype(bf),
        'Wpk': Wpk,
    }

    in_maps = []
    for core in range(8):
        b, h = core // 2, core % 2
        tmap = list(range(16)) if h == 0 else list(range(6, 16)) + list(range(6))
        Xp = X[b][:, :, tmap]                              # (N, F, 16)
        Xn = np.ascontiguousarray(Xp.transpose(0, 2, 1).reshape(N, T * F)).astype(bf)
        Xw = np.ascontiguousarray(Xp.transpose(2, 1, 0).reshape(8, P, N)).astype(bf)
        UW = np.zeros((8, P, 48), np.float32)
        for tp in range(16):
            s, v = tp // 2, tp % 2
            UW[s, 64 * v:64 * v + 64, tp] = Ws3
            UW[s, 64 * v:64 * v + 64, 32 + tp] = U3
        Pb = np.zeros((P, PBW), np.float32)
        Pb[:, 0:4] = U1.reshape(4, P).T
        Pb[:, 4:20] = np.vstack([Ws2, Ws2])
        VeTh = 0.5 * Ve[np.ix_(tmap, tmap)].T
        Pb[0:16, 20:36] = VeTh
        Pb[0:16, 36] = Ws1[tmap]
        Pb[0, 37:165] = 1.0
        Pb[:, 165:293] = np.eye(P, dtype=np.float32)
        Pb[0:64, 293:805] = U2
        Pb[0, 805:1061] = np.eye(T, dtype=np.float32).reshape(-1)
        # B2: (128, 2) block indicator * 1/64 for per-v mean over f
        Pb[0:64, 1061] = 1.0 / 64
        Pb[64:128, 1062] = 1.0 / 64
        # B2T: (2, 128) block indicator for broadcast back, gamma folded in
        # (replicated at partition 32 for the grouped-stats row-32 slices)
        Pb[0, 1063:1127] = gamma
        Pb[1, 1127:1191] = gamma
        Pb[32, 1063:1127] = gamma
        Pb[33, 1127:1191] = gamma
        # sigmoid-fold rank-1 rows
        Pb[0, 1191:1207] = VeTh.sum(axis=0)
        Pb[0, 1207:1719] = vch
        Pb[0:16, 1719:1735] = 0.5 * be[0][np.ix_(tmap, tmap)]
        Pfc = Pf.copy()
        Pfc[0:16, 132:148] = 0.5 * be[0][np.ix_(tmap, tmap)]
        m = dict(shared)
        m.update({
            'Xn': Xn, 'Xw': Xw, 'UW': UW.astype(bf),
            'Pb': Pb.astype(bf), 'Pf': Pfc,
        })
        in_maps.append(m)
    return in_maps


def kernel(**inputs):
    import sys
    if '/opt/trn_rl_repo' not in sys.path:
        sys.path.insert(0, '/opt/trn_rl_repo')
    from concourse.bass_utils import run_bass_kernel_spmd

    if 'nc' not in _CACHE:
        _CACHE['nc'] = _build_program()
    nc = _CACHE['nc']

    in_maps = _host_prep(inputs)
    res = run_bass_kernel_spmd(nc, in_maps, list(range(8)))
    out = np.zeros((B, N, F, T), np.float32)
    for core in range(8):
        b, h = core // 2, core % 2
        Z = np.asarray(res.results[core]['Zout']).astype(np.float32)
        # rows q*128 + v*64 + f, cols n  ->  (n, f, slot=2q+v)
        Zs = Z.reshape(NP, 2, F, N).transpose(3, 2, 0, 1).reshape(N, F, NSLOT)
        wstart = 0 if h == 0 else 6
        jlo = 0 if h == 0 else 2
        out[b, :, :, wstart + jlo:wstart + jlo + 8] = Zs[:, :, jlo:jlo + 8]
    return out
